# revision 1
# baseline (speedup 1.0000x reference)
"""Causal MHA with RoPE on 8 TRN2 NeuronCores.

Sharding: data-parallel over batch (2) x tensor-parallel over heads (4 groups
of 4 heads) = 8 cores. Core c handles batch c//4, head group c%4.
Each core computes its 4 heads' attention and a partial output projection
(Wo sharded row-wise); host sums the 4 partials per batch.

Per-core device algorithm (all matmuls in float32r = TF32, fp32 accumulate):
  - QK^T projection: qkT[dk, s] = (Wqk rows).T-contracted with xT (host-transposed x)
  - RoPE applied on [dk(partition), s] layout via cos/sin tables and a
    stream_shuffle partition pair-swap
  - scores^T[k, q] = K^T.T-free @ Q^T per head (K=64 contraction, two heads
    packed in row groups 0-1 / 2-3 of the PE array)
  - probsT = exp(scores/8) straight from PSUM (no max subtraction; scores are
    N(0,1)-scaled so exp never overflows), causal tri-mask on diagonal tiles
  - attnT_unnorm[dk, q] (+ row of sums via a ones column in [V|1]) = [V|1].T @ probsT
  - softmax normalization: sums row -> K=1 broadcast matmul -> reciprocal ->
    one elementwise multiply
  - partial out = attnT.T-contracted with WoT chunks, accumulated over the
    2 head pairs, DMA'd to DRAM
"""
import sys
import os

for _p in ("/opt/trn_rl_repo", "/root/.axon_site/_ro/trn_rl_repo"):
    if os.path.isdir(_p) and _p not in sys.path:
        sys.path.insert(0, _p)

import numpy as np

import concourse.mybir as mybir
import concourse.tile as tile
from concourse import bacc
from concourse.bass_utils import run_bass_kernel_spmd

F32 = mybir.dt.float32
F32R = mybir.dt.float32r
AF = mybir.ActivationFunctionType
MULT = mybir.AluOpType.mult
ADD = mybir.AluOpType.add

B, S, D = 2, 2048, 1024
H, DK = 16, 64
THETA = 10000.0
NCORES = 8
GROUPS = 4          # head groups (tensor parallel)
GH = H // GROUPS    # heads per group = 4
GF = GH * DK        # features per group = 256
SWAP_MASK = [i ^ 1 for i in range(32)]
KVER = 14  # bump on any kernel change: busts the HLO-shape-keyed NEFF cache

_CACHED = {}


def _build_nc(iters=1):
    _iters = iters
    nc = bacc.Bacc("TRN2", target_bir_lowering=False, debug=False, num_devices=NCORES)
    xT = nc.dram_tensor("xT", [D, S], F32R, kind="ExternalInput").ap()
    wqkT = nc.dram_tensor("wqkT", [D, 2 * GF], F32R, kind="ExternalInput").ap()
    wvT = nc.dram_tensor("wvT", [D, GF], F32R, kind="ExternalInput").ap()
    woT = nc.dram_tensor("woT", [GF, D], F32R, kind="ExternalInput").ap()
    cosf = nc.dram_tensor("cosf", [128, S], F32, kind="ExternalInput").ap()
    sins = nc.dram_tensor("sins", [128, S], F32, kind="ExternalInput").ap()
    tri = nc.dram_tensor("tri", [128, 128], F32, kind="ExternalInput").ap()
    ho = nc.dram_tensor("ho", [1, 256], F32R, kind="ExternalInput").ap()
    onesc = nc.dram_tensor("onesc", [128, 1], F32R, kind="ExternalInput").ap()
    # unused input whose shape encodes the kernel version: the neuron compile
    # cache keys on HLO structure only, so two kernels with identical I/O
    # shapes would otherwise collide.
    nc.dram_tensor("cachebust", [iters, KVER], F32, kind="ExternalInput")
    out = nc.dram_tensor("out", [S, D], F32, kind="ExternalOutput").ap()

    SB = S // 512  # 4 q-tiles of 512
    KB = S // 128  # 16 k-blocks of 128

    with tile.TileContext(nc) as tc:
        with tc.tile_pool(name="const", bufs=1) as cpool, \
             tc.tile_pool(name="xt", bufs=8) as xpool, \
             tc.tile_pool(name="big", bufs=1) as bpool, \
             tc.tile_pool(name="work", bufs=2) as wpool, \
             tc.tile_pool(name="probs", bufs=4) as ppool, \
             tc.tile_pool(name="psum", bufs=1, space="PSUM") as psum:

            # ---- loads, ordered by first use on the single HWDGE queue ----
            wqk_sb = cpool.tile([128, 8, 2 * GF], F32R, tag="wqk")
            wv_sb = cpool.tile([128, 8, GF], F32R, tag="wv")
            wo_sb = cpool.tile([128, 2, D], F32R, tag="wo")
            cos_sb = cpool.tile([128, S], F32, tag="cos")
            sin_sb = cpool.tile([128, S], F32, tag="sin")
            tri_sb = cpool.tile([128, 128], F32, tag="tri")
            ho_sb = cpool.tile([1, 256], F32R, tag="ho")
            onesc_sb = cpool.tile([128, 1], F32R, tag="onesc")
            xt_tiles = []
            for dc in range(8):
                t = xpool.tile([128, S], F32R, tag="xt")
                xt_tiles.append(t)

            for dc in range(8):  # interleaved so MM(dc) unblocks asap
                nc.sync.dma_start(wqk_sb[:, dc, :], wqkT[dc * 128:(dc + 1) * 128, :])
                nc.sync.dma_start(xt_tiles[dc][:, 0:512],
                                  xT[dc * 128:(dc + 1) * 128, 0:512])
            nc.sync.dma_start(cos_sb[:, 0:1024], cosf[:, 0:1024])
            nc.sync.dma_start(sin_sb[:, 0:1024], sins[:, 0:1024])
            nc.sync.dma_start(onesc_sb[:], onesc)
            nc.sync.dma_start(ho_sb[:], ho)
            nc.sync.dma_start(tri_sb[:], tri)
            for dc in range(8):  # t=1 quarter
                nc.sync.dma_start(xt_tiles[dc][:, 512:1024],
                                  xT[dc * 128:(dc + 1) * 128, 512:1024])
            nc.sync.dma_start(wv_sb[:], wvT.rearrange("(dc p) n -> p dc n", p=128))
            nc.sync.dma_start(cos_sb[:, 1024:S], cosf[:, 1024:S])
            nc.sync.dma_start(sin_sb[:, 1024:S], sins[:, 1024:S])
            nc.sync.dma_start(wo_sb[:], woT.rearrange("(fc p) n -> p fc n", p=128))
            for dc in range(8):  # t=2,3 half
                nc.sync.dma_start(xt_tiles[dc][:, 1024:S],
                                  xT[dc * 128:(dc + 1) * 128, 1024:S])

            warm = cpool.tile([1, 1], F32, tag="warm")
            nc.scalar.activation(warm[:], ho_sb[0:1, 0:1], AF.Exp, scale=1.0)

            # ---- phase 1: QK projection + RoPE ----
            # qkT slabs: 0,1 = Q head-pairs; 2,3 = K head-pairs
            for _it in range(iters):
              qkT = bpool.tile([128, 4, S], F32R, tag="qkT", name=f"qkT{_it}")
              vt = bpool.tile([128, KB, GH, DK + 1], F32R, tag="vt", name=f"vt{_it}")
              nc.vector.tensor_copy(
                  vt[:, :, :, DK:DK + 1],
                  onesc_sb[:, None, None, :].to_broadcast([128, KB, GH, 1]))
              for t in range(SB):
                  for c in range(4):
                      ps = psum.tile([128, 512], F32, tag="sc", bufs=2)
                      for dc in range(8):
                          nc.tensor.matmul(
                              ps[:], wqk_sb[:, dc, c * 128:(c + 1) * 128],
                              xt_tiles[dc][:, t * 512:(t + 1) * 512],
                              start=(dc == 0), stop=(dc == 7))
                      tsl = slice(t * 512, (t + 1) * 512)
                      # rope: qkT = ps*cos + swap(ps*sins)
                      nc.vector.tensor_tensor(qkT[:, c, tsl], ps[:], cos_sb[:, tsl], MULT)
                      tmp = wpool.tile([128, 512], F32, tag="ropetmp")
                      nc.vector.tensor_tensor(tmp[:], ps[:], sin_sb[:, tsl], MULT)
                      tmp2 = wpool.tile([128, 512], F32, tag="ropetmp2")
                      nc.vector.stream_shuffle(tmp2[:], tmp[:], SWAP_MASK)
                      nc.gpsimd.tensor_tensor(qkT[:, c, tsl], qkT[:, c, tsl], tmp2[:], ADD)
                  # V projection for this t's 4 s-blocks (interleaved with QK)
                  for sb_i in range(4 * t, 4 * t + 4):
                      psv = psum.tile([128, GF], F32, tag="sc", bufs=2)
                      for dc in range(8):
                          nc.tensor.matmul(
                              psv[:], xt_tiles[dc][:, sb_i * 128:(sb_i + 1) * 128],
                              wv_sb[:, dc, :], start=(dc == 0), stop=(dc == 7))
                      nc.scalar.copy(
                          vt[:, sb_i, :, 0:DK],
                          psv[:].rearrange("p (h d) -> p h d", h=GH))

              # ---- phase 3: attention per head pair ----
              attnT = [cpool.tile([128, S], F32R, tag=("cos" if p == 0 else "sin"),
                                  name=f"attnT{p}_{_it}") for p in range(2)]
              for qt in range(SB):
                  for pair in range(2):
                      qs, ks = pair, 2 + pair
                      pva = psum.tile([128, 512], F32, tag="pv", bufs=2)
                      pvb = psum.tile([128, 512], F32, tag="pv", bufs=2)
                      nkb = 4 * qt + 4
                      for kb in range(nkb):
                          lam = max(kb - 4 * qt, 0) * 128
                          qsl = slice(qt * 512 + lam, (qt + 1) * 512)
                          ksl = slice(kb * 128, (kb + 1) * 128)
                          ss = psum.tile([128, 2, 512], F32, tag="sc2", bufs=2)
                          nc.tensor.matmul(ss[:, 0, lam:512], qkT[0:64, ks, ksl],
                                           qkT[0:64, qs, qsl], start=True, stop=True)
                          nc.tensor.matmul(ss[:, 1, lam:512], qkT[64:128, ks, ksl],
                                           qkT[64:128, qs, qsl], start=True, stop=True)
                          pab = ppool.tile([128, 2, 512], F32R, tag="probs")
                          nc.scalar.activation(pab[:, :, lam:512], ss[:, :, lam:512], AF.Exp, scale=0.125)
                          if kb >= 4 * qt:  # diagonal block: causal tri mask
                              dsl = slice(lam, lam + 128)
                              nc.gpsimd.tensor_tensor(
                                  pab[:, :, dsl], pab[:, :, dsl],
                                  tri_sb[:, None, :].to_broadcast([128, 2, 128]), MULT)
                          nc.tensor.matmul(pva[0:65, lam:512], vt[:, kb, 2 * pair, :],
                                           pab[:, 0, lam:512], start=(kb == 0), stop=(kb == nkb - 1))
                          nc.tensor.matmul(pvb[0:65, lam:512], vt[:, kb, 2 * pair + 1, :],
                                           pab[:, 1, lam:512], start=(kb == 0), stop=(kb == nkb - 1))
                      # normalization: sums row 64 -> bcast -> recip -> multiply
                      qtsl = slice(qt * 512, (qt + 1) * 512)
                      sra = wpool.tile([1, 512], F32R, tag="srow")
                      srb = wpool.tile([1, 512], F32R, tag="srow2")
                      nc.vector.tensor_copy(sra[:], pva[64:65, :])
                      nc.vector.tensor_copy(srb[:], pvb[64:65, :])
                      psr = psum.tile([128, 512], F32, tag="sc", bufs=2)
                      nc.tensor.matmul(psr[:], ho_sb[:, 0:128], sra[:], start=True, stop=False)
                      nc.tensor.matmul(psr[:], ho_sb[:, 128:256], srb[:], start=False, stop=True)
                      rbc = wpool.tile([128, 512], F32, tag="rbc_sb")
                      nc.vector.reciprocal(rbc[:], psr[:])
                      nc.vector.tensor_copy(attnT[pair][0:64, qtsl], pva[0:64, :])
                      nc.vector.tensor_copy(attnT[pair][64:128, qtsl], pvb[0:64, :])
                      nc.vector.tensor_tensor(attnT[pair][:, qtsl], attnT[pair][:, qtsl],
                                              rbc[:], MULT)

                  # ---- output projection for this qt's q-blocks (interleaved) ----
                  for qb in range(4 * qt, 4 * qt + 4):
                      qsl = slice(qb * 128, (qb + 1) * 128)
                      osb = xpool.tile([128, D], F32, tag="xt", name=f"osb{qb}_{_it}")
                      for nh in range(2):
                          nsl = slice(nh * 512, (nh + 1) * 512)
                          pso = psum.tile([128, 512], F32, tag="sc", bufs=2)
                          nc.tensor.matmul(pso[:], attnT[0][:, qsl], wo_sb[:, 0, nsl],
                                           start=True, stop=False)
                          nc.tensor.matmul(pso[:], attnT[1][:, qsl], wo_sb[:, 1, nsl],
                                           start=False, stop=True)
                          nc.vector.tensor_copy(osb[:, nsl], pso[:])
                      nc.sync.dma_start(out[qsl, :], osb[:])

    nc.compile()
    return nc


def _host_tables(token_positions):
    pos = np.asarray(token_positions, dtype=np.float32)  # [S]
    half = DK // 2
    freq = THETA ** (-np.arange(0, DK, 2, dtype=np.float32) / DK)  # [32]
    # per-partition tables on [dk(128 = 2 heads of 64), s]
    f64 = np.repeat(freq, 2)          # [64] freq per feature index
    ang64 = pos[None, :] * f64[:, None]  # [64, S]
    cos64 = np.cos(ang64)
    sin64 = np.sin(ang64)
    sign = np.where(np.arange(DK) % 2 == 0, 1.0, -1.0).astype(np.float32)  # +s even, -s odd
    sins64 = sin64 * sign[:, None]
    cosf = np.concatenate([cos64, cos64], axis=0).astype(np.float32)   # [128, S]
    sins = np.concatenate([sins64, sins64], axis=0).astype(np.float32)  # [128, S]
    return cosf, sins


def kernel(x, Wq, Wk, Wv, Wo, token_positions):
    x = np.asarray(x, dtype=np.float32)
    Wq = np.asarray(Wq, dtype=np.float32)
    Wk = np.asarray(Wk, dtype=np.float32)
    Wv = np.asarray(Wv, dtype=np.float32)
    Wo = np.asarray(Wo, dtype=np.float32)

    if "nc" not in _CACHED:
        _CACHED["nc"] = _build_nc(iters=int(os.environ.get("BENCH_ITERS", "1")))
    nc = _CACHED["nc"]

    cosf, sins = _host_tables(token_positions)
    tri = np.triu(np.ones((128, 128), dtype=np.float32))  # tri[k, j] = 1 if j >= k
    ho = np.concatenate([
        np.concatenate([np.ones(64), np.zeros(64)]),
        np.concatenate([np.zeros(64), np.ones(64)]),
    ]).astype(np.float32)[None, :]  # [1, 256]
    onesc = np.ones((128, 1), dtype=np.float32)

    xT = [np.ascontiguousarray(x[b].T) for b in range(B)]  # [D, S] each
    in_maps = []
    for c in range(NCORES):
        b, g = c // GROUPS, c % GROUPS
        R = slice(g * GF, (g + 1) * GF)
        wqkT = np.ascontiguousarray(
            np.concatenate([Wq[R].T, Wk[R].T], axis=1))  # [D, 512]
        wvT = np.ascontiguousarray(Wv[R].T)              # [D, 256]
        woT = np.ascontiguousarray(Wo[:, R].T)           # [256, D]
        in_maps.append({
            "xT": xT[b], "wqkT": wqkT, "wvT": wvT, "woT": woT,
            "cosf": cosf, "sins": sins, "tri": tri, "ho": ho, "onesc": onesc,
            "cachebust": np.zeros((int(os.environ.get("BENCH_ITERS", "1")), KVER), dtype=np.float32),
        })

    try:
        res = run_bass_kernel_spmd(nc, in_maps, core_ids=list(range(NCORES)))
    except Exception:
        # transient NRT_EXEC_UNIT_UNRECOVERABLE flakes recover on retry
        import time as _time
        _time.sleep(2.0)
        res = run_bass_kernel_spmd(nc, in_maps, core_ids=list(range(NCORES)))
    _CACHED["last_results"] = res
    outs = [r["out"] for r in res.results]  # each [S, D] partial
    full = np.empty((B, S, D), dtype=np.float32)
    for b in range(B):
        full[b] = sum(outs[b * GROUPS + g] for g in range(GROUPS))
    return full



# revision 54
# speedup vs baseline: 1.2203x; 1.2203x over previous
"""Causal MHA with RoPE on 8 TRN2 NeuronCores.

Sharding: data-parallel over batch (2) x tensor-parallel over heads (4 groups
of 4 heads) = 8 cores. Core c handles batch c//4, head group c%4.
Each core computes its 4 heads' attention and a partial output projection
(Wo sharded row-wise); host sums the 4 partials per batch.

Per-core device algorithm (all matmuls in float32r = TF32, fp32 accumulate):
  - QK^T projection: qkT[dk, s] = (Wqk rows).T-contracted with xT (host-transposed x)
  - RoPE applied on [dk(partition), s] layout via cos/sin tables and a
    stream_shuffle partition pair-swap
  - scores^T[k, q] = K^T.T-free @ Q^T per head (K=64 contraction, two heads
    packed in row groups 0-1 / 2-3 of the PE array)
  - probsT = exp(scores/8) straight from PSUM (no max subtraction; scores are
    N(0,1)-scaled so exp never overflows), causal tri-mask on diagonal tiles
  - attnT_unnorm[dk, q] (+ row of sums via a ones column in [V|1]) = [V|1].T @ probsT
  - softmax normalization: sums row -> K=1 broadcast matmul -> reciprocal ->
    one elementwise multiply
  - partial out = attnT.T-contracted with WoT chunks, accumulated over the
    2 head pairs, DMA'd to DRAM
"""
import sys
import os

for _p in ("/opt/trn_rl_repo", "/root/.axon_site/_ro/trn_rl_repo"):
    if os.path.isdir(_p) and _p not in sys.path:
        sys.path.insert(0, _p)

import numpy as np

import concourse.mybir as mybir
import concourse.tile as tile
from concourse import bacc
from concourse.bass_utils import run_bass_kernel_spmd

F32 = mybir.dt.float32
F32R = mybir.dt.float32r
BF16 = mybir.dt.bfloat16
AF = mybir.ActivationFunctionType
MULT = mybir.AluOpType.mult
ADD = mybir.AluOpType.add
DIV = mybir.AluOpType.divide

B, S, D = 2, 2048, 1024
H, DK = 16, 64
THETA = 10000.0
NCORES = 8
GROUPS = 4          # head groups (tensor parallel)
GH = H // GROUPS    # heads per group = 4
GF = GH * DK        # features per group = 256
SWAP_MASK = [i ^ 1 for i in range(32)]
KVER = 16  # bump on any kernel change: busts the HLO-shape-keyed NEFF cache

_CACHED = {}


def _build_nc(iters=1):
    _iters = iters
    nc = bacc.Bacc("TRN2", target_bir_lowering=False, debug=False, num_devices=NCORES)
    xT = nc.dram_tensor("xT", [D, S], BF16, kind="ExternalInput").ap()
    wqkT = nc.dram_tensor("wqkT", [D, 2 * GF], BF16, kind="ExternalInput").ap()
    wvT = nc.dram_tensor("wvT", [D, GF], BF16, kind="ExternalInput").ap()
    woT = nc.dram_tensor("woT", [GF, D], BF16, kind="ExternalInput").ap()
    cosf = nc.dram_tensor("cosf", [128, S], BF16, kind="ExternalInput").ap()
    sins = nc.dram_tensor("sins", [128, S], BF16, kind="ExternalInput").ap()
    tri = nc.dram_tensor("tri", [128, 128], BF16, kind="ExternalInput").ap()
    ident = nc.dram_tensor("ident", [128, 128], BF16, kind="ExternalInput").ap()
    onesc = nc.dram_tensor("onesc", [128, 1], F32R, kind="ExternalInput").ap()
    # unused input whose shape encodes the kernel version: the neuron compile
    # cache keys on HLO structure only, so two kernels with identical I/O
    # shapes would otherwise collide.
    nc.dram_tensor("cachebust", [iters, KVER], F32, kind="ExternalInput")
    out = nc.dram_tensor("out", [S, D], BF16, kind="ExternalOutput").ap()

    SB = S // 512  # 4 q-tiles of 512
    KB = S // 128  # 16 k-blocks of 128

    with tile.TileContext(nc) as tc:
        with tc.tile_pool(name="const", bufs=1) as cpool, \
             tc.tile_pool(name="big", bufs=1) as bpool, \
             tc.tile_pool(name="work", bufs=2) as wpool, \
             tc.tile_pool(name="asb", bufs=3) as apool, \
             tc.tile_pool(name="probs", bufs=4) as ppool, \
             tc.tile_pool(name="obuf", bufs=3) as opool, \
             tc.tile_pool(name="psum", bufs=1, space="PSUM") as psum:

            # ---- loads, ordered by first use on the single HWDGE queue ----
            wqk_sb = cpool.tile([128, 8, 2 * GF], BF16, tag="wqk")
            wv_sb = cpool.tile([128, 8, GF], BF16, tag="wv")
            wo_sb = cpool.tile([128, 2, D], BF16, tag="wo")
            cos_sb = cpool.tile([128, S], BF16, tag="cos")
            sin_sb = cpool.tile([128, S], BF16, tag="sin")
            tri_sb = cpool.tile([128, 128], BF16, tag="tri")
            ident_sb = cpool.tile([128, 128], BF16, tag="ident")
            onesc_sb = cpool.tile([128, 1], F32R, tag="onesc")
            xt_all = cpool.tile([128, 8, S], BF16, tag="xt")

            def xt_load(tsl):
                # x feature-chunks dc packed on partitions, batched in 2 DMAs
                for half in range(2):
                    nc.sync.dma_start(
                        xt_all[:, half * 4:(half + 1) * 4, tsl],
                        xT[half * 512:(half + 1) * 512, tsl].rearrange(
                            "(dc p) s -> p dc s", p=128))

            def wqk_load(c):
                nc.sync.dma_start(
                    wqk_sb[:, :, c * 128:(c + 1) * 128],
                    wqkT[:, c * 128:(c + 1) * 128].rearrange(
                        "(dc p) n -> p dc n", p=128))

            wqk_load(0)
            xt_load(slice(0, 512))
            wqk_load(2)
            nc.sync.dma_start(wv_sb[:], wvT.rearrange("(dc p) n -> p dc n", p=128))
            nc.sync.dma_start(onesc_sb[:], onesc)
            nc.sync.dma_start(cos_sb[:, 0:1024], cosf[:, 0:1024])
            nc.sync.dma_start(sin_sb[:, 0:1024], sins[:, 0:1024])
            wqk_load(1)
            wqk_load(3)
            nc.sync.dma_start(tri_sb[:], tri)
            xt_load(slice(512, 1024))
            nc.sync.dma_start(ident_sb[:], ident)
            nc.sync.dma_start(cos_sb[:, 1024:S], cosf[:, 1024:S])
            nc.sync.dma_start(sin_sb[:, 1024:S], sins[:, 1024:S])
            nc.sync.dma_start(wo_sb[:], woT.rearrange("(fc p) n -> p fc n", p=128))
            xt_load(slice(1024, S))

            warm = cpool.tile([1, 1], F32, tag="warm")
            nc.scalar.activation(warm[:], onesc_sb[0:1, 0:1], AF.Exp, scale=1.0)

            # ---- kernel body ----
            # Projections (phase 1) for x-tile t+1 are interleaved INTO the
            # attention kb-loop over qt=t: attention is ACT(exp)-bound, so the
            # PE fills its idle slots with the next tile's QK/V projections.
            for _it in range(iters):
              qkT = bpool.tile([128, 4, S], BF16, tag="qkT", name=f"qkT{_it}")
              vt = bpool.tile([128, KB, GH, DK + 1], BF16, tag="vt", name=f"vt{_it}")
              nc.vector.tensor_copy(
                  vt[:, :, :, DK:DK + 1],
                  onesc_sb[:, None, None, :].to_broadcast([128, KB, GH, 1]))

              def proj_qk(t, c):
                  # QK projection chunk: 128 features (head pair c of Q|K),
                  # 512 seq positions, full D contraction; then RoPE.
                  ps = psum.tile([128, 512], F32, tag="sc", bufs=2)
                  for dc in range(8):
                      nc.tensor.matmul(
                          ps[:], wqk_sb[:, dc, c * 128:(c + 1) * 128],
                          xt_all[:, dc, t * 512:(t + 1) * 512],
                          start=(dc == 0), stop=(dc == 7))
                  tsl = slice(t * 512, (t + 1) * 512)
                  # rope: qkT = ps*cos + swap(ps*sins), all on DVE (Pool is
                  # reserved for the latency-critical causal masks)
                  tmp = wpool.tile([128, 512], BF16, tag="ropetmp")
                  nc.vector.tensor_tensor(tmp[:], ps[:], sin_sb[:, tsl], MULT)
                  tmp2 = wpool.tile([128, 512], BF16, tag="ropetmp2")
                  nc.vector.stream_shuffle(tmp2[:], tmp[:], SWAP_MASK)
                  nc.vector.tensor_tensor(qkT[:, c, tsl], ps[:], cos_sb[:, tsl], MULT)
                  nc.vector.tensor_tensor(qkT[:, c, tsl], qkT[:, c, tsl], tmp2[:], ADD)

              def proj_v(sb_i):
                  psv = psum.tile([128, GF], F32, tag="sc", bufs=2)
                  for dc in range(8):
                      nc.tensor.matmul(
                          psv[:], xt_all[:, dc, sb_i * 128:(sb_i + 1) * 128],
                          wv_sb[:, dc, :], start=(dc == 0), stop=(dc == 7))
                  nc.vector.tensor_copy(
                      vt[:, sb_i, :, 0:DK],
                      psv[:].rearrange("p (h d) -> p h d", h=GH))

              # Deferred-PE-work queue: projection chunks for tile t+1,
              # transposes of the previous pair, and the previous qt's output
              # projection all get pumped into the attention kb-loop so the
              # (in-order) PE never sits behind a dependency-stalled
              # instruction for long.
              from collections import deque
              fill_q = deque()   # prompt PE work (projections, transposes)
              late_q = deque()   # output projections, deferred to late qts
                                 # where attention has an ACT-vs-PE deficit

              def pump(n=1, late_ok=False):
                  for _ in range(n):
                      if fill_q:
                          fill_q.popleft()()
                      elif late_ok and late_q:
                          late_q.popleft()()

              def transpose_unit(gq, pair, att_sb, qb):
                  def run():
                      tps = psum.tile([128, 128], BF16, tag="sc", bufs=2,
                                      name=f"tps{gq}_{pair}_{_it}")
                      nc.tensor.transpose(tps[:], att_sb[:, qb, :], ident_sb[:])
                      nc.vector.tensor_copy(
                          attnT[pair][:, gq * 128:(gq + 1) * 128], tps[:])
                  return run

              def oproj_unit(qb, split_copy=False):
                  # output projection for one 128-q-block
                  def run():
                      qsl = slice(qb * 128, (qb + 1) * 128)
                      osb = opool.tile([128, D], BF16, tag="osb",
                                       name=f"osb{qb}_{_it}")
                      for nh in range(2):
                          nsl = slice(nh * 512, (nh + 1) * 512)
                          pso = psum.tile([128, 512], F32, tag="sc", bufs=2)
                          nc.tensor.matmul(pso[:], attnT[0][:, qsl],
                                           wo_sb[:, 0, nsl], start=True, stop=False)
                          nc.tensor.matmul(pso[:], attnT[1][:, qsl],
                                           wo_sb[:, 1, nsl], start=False, stop=True)
                          if split_copy and nh == 1:
                              nc.scalar.copy(osb[:, nsl], pso[:])
                          else:
                              nc.vector.tensor_copy(osb[:, nsl], pso[:])
                      nc.sync.dma_start(out[qsl, :], osb[:])
                  return run

              # ---- attention (PV-flipped), deferred work interleaved ----
              # PV: attn[q, dk] = probsT.T @ [V|1] per 128-q-block: N=65 moving
              # rows instead of N=512, fully using the 128-wide K (k-positions)
              # and M (q) dims of the PE array. Softmax sums land in column 64
              # as per-partition scalars -> normalization via Pool broadcast
              # multiply, then a PE transpose restores [f, q] layout for the
              # output projection.
              attnT = [bpool.tile([128, S], BF16, tag=f"attnT{p}",
                                  name=f"attnT{p}_{_it}") for p in range(2)]

              # tile t=0: pair-0's needs (Q01, K01, V) up front; Q23/K23 queued
              proj_qk(0, 0)
              proj_qk(0, 2)
              for s in range(4):
                  proj_v(s)
              fill_q.append(lambda: proj_qk(0, 1))
              fill_q.append(lambda: proj_qk(0, 3))

              for qt in range(SB):
                  if qt + 1 < SB:
                      t = qt + 1
                      for c in range(4):
                          fill_q.append(lambda t=t, c=c: proj_qk(t, c))
                      for s in range(4 * t, 4 * t + 4):
                          fill_q.append(lambda s=s: proj_v(s))
                  nkb = 4 * qt + 4
                  nsteps = 2 * nkb
                  step = 0
                  for pair in range(2):
                      qs, ks = pair, 2 + pair
                      attps = [psum.tile([128, 4, DK + 1], F32, tag="att", bufs=2,
                                         name=f"attps{h}_{qt}_{pair}_{_it}")
                               for h in range(2)]
                      # Interleaved accumulation GROUPS in one PSUM bank are
                      # broken on HW (start=True wipes the bank), so zero the
                      # bank once and accumulate with start=False throughout.
                      for h in range(2):
                          nc.vector.memset(attps[h][:], 0.0)

                      def pv(kb):
                          for qb in range(max(kb - 4 * qt, 0), 4):
                              gq = 4 * qt + qb
                              for h in range(2):
                                  nc.tensor.matmul(
                                      attps[h][:, qb, :],
                                      pab_ring[kb % 4][:, h, qb * 128:(qb + 1) * 128],
                                      vt[:, kb, 2 * pair + h, :],
                                      start=False, stop=(kb == gq),
                                      skip_group_check=True)

                      att_sb = apool.tile([128, 4, 128], BF16, tag="attsb")
                      rsum = wpool.tile([128, 4, 2], F32, tag="rsum")
                      last = qt == SB - 1 and pair == 1

                      def normalize(qb):
                          # DVE reciprocal of the col-64 sums + DVE broadcast
                          # multiply (gpsimd cannot read PSUM). Per-q-block
                          # only for the very last pair (to pipeline the
                          # tail); batched per-pair otherwise to keep the DVE
                          # instruction count down. Transposes go to the FRONT
                          # of the queue: they are small, release the shared
                          # "sc" PSUM ring fast, and feed the output
                          # projection.
                          if not last:
                              if qb < 3:
                                  return
                              qsl3, nq = slice(0, 4), 4
                          else:
                              qsl3, nq = slice(qb, qb + 1), 1
                          for h in range(2):
                              nc.vector.reciprocal(rsum[:, qsl3, h:h + 1],
                                                   attps[h][:, qsl3, DK:DK + 1])
                              nc.vector.tensor_tensor(
                                  att_sb[:, qsl3, h * 64:(h + 1) * 64],
                                  attps[h][:, qsl3, 0:DK],
                                  rsum[:, qsl3, h:h + 1].to_broadcast(
                                      [128, nq, DK]), MULT)
                          if last:
                              # last qt: output projection chases each q-block
                              fill_q.appendleft(
                                  oproj_unit(4 * qt + qb, split_copy=True))
                              fill_q.appendleft(
                                  transpose_unit(4 * qt + qb, pair, att_sb, qb))
                          else:
                              for b in range(3, -1, -1):
                                  fill_q.appendleft(
                                      transpose_unit(4 * qt + b, pair, att_sb, b))

                      pab_ring = {}
                      for kb in range(nkb):
                          lam = max(kb - 4 * qt, 0) * 128
                          qsl = slice(qt * 512 + lam, (qt + 1) * 512)
                          ksl = slice(kb * 128, (kb + 1) * 128)
                          ss = psum.tile([128, 2, 512], F32, tag="sc2", bufs=2)
                          nc.tensor.matmul(ss[:, 0, lam:512], qkT[0:64, ks, ksl],
                                           qkT[0:64, qs, qsl], start=True, stop=True)
                          nc.tensor.matmul(ss[:, 1, lam:512], qkT[64:128, ks, ksl],
                                           qkT[64:128, qs, qsl], start=True, stop=True)
                          pab = ppool.tile([128, 2, 512], BF16, tag="probs")
                          pab_ring[kb % 4] = pab
                          nc.scalar.activation(pab[:, :, lam:512], ss[:, :, lam:512], AF.Exp, scale=0.125)
                          if kb >= 4 * qt:  # diagonal block: causal tri mask
                              dsl = slice(lam, lam + 128)
                              nc.gpsimd.tensor_tensor(
                                  pab[:, :, dsl], pab[:, :, dsl],
                                  tri_sb[:, None, :].to_broadcast([128, 2, 128]), MULT)
                          # drain queued PE work evenly across this qt's steps;
                          # late (output-projection) work backfills in the
                          # ACT-deficit qts
                          n = -(-len(fill_q) // (nsteps - step)) if fill_q else 0
                          if qt >= SB - 2:
                              n = max(n, 1)
                          pump(n, late_ok=(qt >= SB - 2))
                          step += 1
                          # software pipeline: PV for the previous kb runs
                          # after this kb's scores are already in flight
                          if kb > 0:
                              pv(kb - 1)
                              if kb - 1 >= 4 * qt:  # that region just stopped
                                  normalize(kb - 1 - 4 * qt)
                      pv(nkb - 1)
                      normalize(3)
                  if qt < SB - 1:
                      for qb in range(4 * qt, 4 * qt + 4):
                          late_q.append(oproj_unit(qb))
              while late_q:
                  late_q.popleft()()
              while fill_q:
                  fill_q.popleft()()

    nc.compile()
    return nc


def _host_tables(token_positions):
    pos = np.asarray(token_positions, dtype=np.float32)  # [S]
    half = DK // 2
    freq = THETA ** (-np.arange(0, DK, 2, dtype=np.float32) / DK)  # [32]
    # per-partition tables on [dk(128 = 2 heads of 64), s]
    f64 = np.repeat(freq, 2)          # [64] freq per feature index
    ang64 = pos[None, :] * f64[:, None]  # [64, S]
    cos64 = np.cos(ang64)
    sin64 = np.sin(ang64)
    sign = np.where(np.arange(DK) % 2 == 0, 1.0, -1.0).astype(np.float32)  # +s even, -s odd
    sins64 = sin64 * sign[:, None]
    from ml_dtypes import bfloat16 as bf16
    cosf = np.concatenate([cos64, cos64], axis=0).astype(bf16)   # [128, S]
    sins = np.concatenate([sins64, sins64], axis=0).astype(bf16)  # [128, S]
    return cosf, sins


def kernel(x, Wq, Wk, Wv, Wo, token_positions):
    from ml_dtypes import bfloat16 as bf16
    x = np.asarray(x, dtype=np.float32)
    Wq = np.asarray(Wq, dtype=np.float32)
    Wk = np.asarray(Wk, dtype=np.float32)
    Wv = np.asarray(Wv, dtype=np.float32)
    Wo = np.asarray(Wo, dtype=np.float32)

    if "nc" not in _CACHED:
        _CACHED["nc"] = _build_nc(iters=int(os.environ.get("BENCH_ITERS", "1")))
    nc = _CACHED["nc"]

    cosf, sins = _host_tables(token_positions)
    tri = np.triu(np.ones((128, 128), dtype=bf16))  # tri[k, j] = 1 if j >= k
    ident = np.eye(128, dtype=bf16)
    onesc = np.ones((128, 1), dtype=np.float32)

    xT = [np.ascontiguousarray(x[b].T).astype(bf16) for b in range(B)]  # [D, S]
    in_maps = []
    for c in range(NCORES):
        b, g = c // GROUPS, c % GROUPS
        R = slice(g * GF, (g + 1) * GF)
        wqkT = np.ascontiguousarray(
            np.concatenate([Wq[R].T, Wk[R].T], axis=1)).astype(bf16)  # [D, 512]
        wvT = np.ascontiguousarray(Wv[R].T).astype(bf16)              # [D, 256]
        woT = np.ascontiguousarray(Wo[:, R].T).astype(bf16)           # [256, D]
        in_maps.append({
            "xT": xT[b], "wqkT": wqkT, "wvT": wvT, "woT": woT,
            "cosf": cosf, "sins": sins, "tri": tri, "ident": ident, "onesc": onesc,
            "cachebust": np.zeros((int(os.environ.get("BENCH_ITERS", "1")), KVER), dtype=np.float32),
        })

    try:
        res = run_bass_kernel_spmd(nc, in_maps, core_ids=list(range(NCORES)))
    except Exception:
        # transient NRT_EXEC_UNIT_UNRECOVERABLE flakes recover on retry
        import time as _time
        _time.sleep(2.0)
        res = run_bass_kernel_spmd(nc, in_maps, core_ids=list(range(NCORES)))
    _CACHED["last_results"] = res
    outs = [np.asarray(r["out"], dtype=np.float32) for r in res.results]  # [S, D]
    full = np.empty((B, S, D), dtype=np.float32)
    for b in range(B):
        full[b] = sum(outs[b * GROUPS + g] for g in range(GROUPS))
    return full



# revision 69
# speedup vs baseline: 1.2464x; 1.0215x over previous
"""Causal MHA with RoPE on 8 TRN2 NeuronCores.

Sharding: data-parallel over batch (2) x tensor-parallel over heads (4 groups
of 4 heads) = 8 cores. Core c handles batch c//4, head group c%4.
Each core computes its 4 heads' attention and a partial output projection
(Wo sharded row-wise); host sums the 4 partials per batch.

Per-core device algorithm (all matmuls in float32r = TF32, fp32 accumulate):
  - QK^T projection: qkT[dk, s] = (Wqk rows).T-contracted with xT (host-transposed x)
  - RoPE applied on [dk(partition), s] layout via cos/sin tables and a
    stream_shuffle partition pair-swap
  - scores^T[k, q] = K^T.T-free @ Q^T per head (K=64 contraction, two heads
    packed in row groups 0-1 / 2-3 of the PE array)
  - probsT = exp(scores/8) straight from PSUM (no max subtraction; scores are
    N(0,1)-scaled so exp never overflows), causal tri-mask on diagonal tiles
  - attnT_unnorm[dk, q] (+ row of sums via a ones column in [V|1]) = [V|1].T @ probsT
  - softmax normalization: sums row -> K=1 broadcast matmul -> reciprocal ->
    one elementwise multiply
  - partial out = attnT.T-contracted with WoT chunks, accumulated over the
    2 head pairs, DMA'd to DRAM
"""
import sys
import os

for _p in ("/opt/trn_rl_repo", "/root/.axon_site/_ro/trn_rl_repo"):
    if os.path.isdir(_p) and _p not in sys.path:
        sys.path.insert(0, _p)

import numpy as np

import concourse.mybir as mybir
import concourse.tile as tile
from concourse import bacc
from concourse.bass_utils import run_bass_kernel_spmd

F32 = mybir.dt.float32
F32R = mybir.dt.float32r
BF16 = mybir.dt.bfloat16
AF = mybir.ActivationFunctionType
MULT = mybir.AluOpType.mult
ADD = mybir.AluOpType.add
DIV = mybir.AluOpType.divide

B, S, D = 2, 2048, 1024
H, DK = 16, 64
THETA = 10000.0
NCORES = 8
GROUPS = 4          # head groups (tensor parallel)
GH = H // GROUPS    # heads per group = 4
GF = GH * DK        # features per group = 256
SWAP_MASK = [i ^ 1 for i in range(32)]
KVER = 16  # bump on any kernel change: busts the HLO-shape-keyed NEFF cache

_CACHED = {}


def _build_nc(iters=1):
    _iters = iters
    nc = bacc.Bacc("TRN2", target_bir_lowering=False, debug=False, num_devices=NCORES)
    xT = nc.dram_tensor("xT", [D, S], BF16, kind="ExternalInput").ap()
    wqkT = nc.dram_tensor("wqkT", [D, 2 * GF], BF16, kind="ExternalInput").ap()
    wvT = nc.dram_tensor("wvT", [D, GF], BF16, kind="ExternalInput").ap()
    woT = nc.dram_tensor("woT", [GF, D], BF16, kind="ExternalInput").ap()
    cosf = nc.dram_tensor("cosf", [128, S], BF16, kind="ExternalInput").ap()
    sins = nc.dram_tensor("sins", [128, S], BF16, kind="ExternalInput").ap()
    tri = nc.dram_tensor("tri", [128, 128], BF16, kind="ExternalInput").ap()
    ident = nc.dram_tensor("ident", [128, 128], BF16, kind="ExternalInput").ap()
    onesc = nc.dram_tensor("onesc", [128, 1], F32R, kind="ExternalInput").ap()
    # unused input whose shape encodes the kernel version: the neuron compile
    # cache keys on HLO structure only, so two kernels with identical I/O
    # shapes would otherwise collide.
    nc.dram_tensor("cachebust", [iters, KVER], F32, kind="ExternalInput")
    out = nc.dram_tensor("out", [S, D], BF16, kind="ExternalOutput").ap()

    SB = S // 512  # 4 q-tiles of 512
    KB = S // 128  # 16 k-blocks of 128

    with tile.TileContext(nc) as tc:
        with tc.tile_pool(name="const", bufs=1) as cpool, \
             tc.tile_pool(name="big", bufs=1) as bpool, \
             tc.tile_pool(name="work", bufs=2) as wpool, \
             tc.tile_pool(name="asb", bufs=3) as apool, \
             tc.tile_pool(name="probs", bufs=4) as ppool, \
             tc.tile_pool(name="obuf", bufs=3) as opool, \
             tc.tile_pool(name="psum", bufs=1, space="PSUM") as psum:

            # ---- loads, ordered by first use on the single HWDGE queue ----
            wqk_sb = cpool.tile([128, 8, 2 * GF], BF16, tag="wqk")
            wv_sb = cpool.tile([128, 8, GF], BF16, tag="wv")
            wo_sb = cpool.tile([128, 2, D], BF16, tag="wo")
            cos_sb = cpool.tile([128, S], BF16, tag="cos")
            sin_sb = cpool.tile([128, S], BF16, tag="sin")
            tri_sb = cpool.tile([128, 128], BF16, tag="tri")
            ident_sb = cpool.tile([128, 128], BF16, tag="ident")
            onesc_sb = cpool.tile([128, 1], F32R, tag="onesc")
            xt_all = cpool.tile([128, 8, S], BF16, tag="xt")

            def xt_load(tsl, nway=2):
                # x feature-chunks dc packed on partitions, batched DMAs
                w = 8 // nway
                for i in range(nway):
                    nc.sync.dma_start(
                        xt_all[:, i * w:(i + 1) * w, tsl],
                        xT[i * w * 128:(i + 1) * w * 128, tsl].rearrange(
                            "(dc p) s -> p dc s", p=128))

            def wqk_load(c):
                nc.sync.dma_start(
                    wqk_sb[:, :, c * 128:(c + 1) * 128],
                    wqkT[:, c * 128:(c + 1) * 128].rearrange(
                        "(dc p) n -> p dc n", p=128))

            wqk_load(0)
            xt_load(slice(0, 512), nway=4)
            nc.sync.dma_start(wv_sb[:], wvT.rearrange("(dc p) n -> p dc n", p=128))
            wqk_load(2)
            nc.sync.dma_start(onesc_sb[:], onesc)
            nc.sync.dma_start(cos_sb[:, 0:1024], cosf[:, 0:1024])
            nc.sync.dma_start(sin_sb[:, 0:1024], sins[:, 0:1024])
            wqk_load(1)
            wqk_load(3)
            nc.sync.dma_start(tri_sb[:], tri)
            xt_load(slice(512, 1024))
            nc.sync.dma_start(ident_sb[:], ident)
            nc.sync.dma_start(cos_sb[:, 1024:S], cosf[:, 1024:S])
            nc.sync.dma_start(sin_sb[:, 1024:S], sins[:, 1024:S])
            nc.sync.dma_start(wo_sb[:], woT.rearrange("(fc p) n -> p fc n", p=128))
            xt_load(slice(1024, S))

            warm = cpool.tile([1, 1], F32, tag="warm")
            nc.scalar.activation(warm[:], onesc_sb[0:1, 0:1], AF.Exp, scale=1.0)

            # ---- kernel body ----
            # Projections (phase 1) for x-tile t+1 are interleaved INTO the
            # attention kb-loop over qt=t: attention is ACT(exp)-bound, so the
            # PE fills its idle slots with the next tile's QK/V projections.
            for _it in range(iters):
              qkT = bpool.tile([128, 4, S], BF16, tag="qkT", name=f"qkT{_it}")
              vt = bpool.tile([128, KB, GH, DK + 1], BF16, tag="vt", name=f"vt{_it}")
              nc.vector.tensor_copy(
                  vt[:, :, :, DK:DK + 1],
                  onesc_sb[:, None, None, :].to_broadcast([128, KB, GH, 1]))

              def proj_qk(t, c, fast=False):
                  # QK projection chunk: 128 features (head pair c of Q|K),
                  # 512 seq positions, full D contraction; then RoPE.
                  ps = psum.tile([128, 512], F32, tag="sc", bufs=2)
                  for dc in range(8):
                      nc.tensor.matmul(
                          ps[:], wqk_sb[:, dc, c * 128:(c + 1) * 128],
                          xt_all[:, dc, t * 512:(t + 1) * 512],
                          start=(dc == 0), stop=(dc == 7))
                  tsl = slice(t * 512, (t + 1) * 512)
                  # rope: qkT = ps*cos + swap(ps*sins), all on DVE (Pool is
                  # reserved for the latency-critical causal masks)
                  if fast:
                      # prologue chunks: pre-round ps to bf16 on the (idle)
                      # ACT engine so the DVE multiplies run in 2x mode
                      psb = wpool.tile([128, 512], BF16, tag="psb")
                      nc.scalar.copy(psb[:], ps[:])
                      src = psb
                  else:
                      src = ps
                  tmp = wpool.tile([128, 512], BF16, tag="ropetmp")
                  nc.vector.tensor_tensor(tmp[:], src[:], sin_sb[:, tsl], MULT)
                  tmp2 = wpool.tile([128, 512], BF16, tag="ropetmp2")
                  nc.vector.stream_shuffle(tmp2[:], tmp[:], SWAP_MASK)
                  nc.vector.tensor_tensor(qkT[:, c, tsl], src[:], cos_sb[:, tsl], MULT)
                  nc.vector.tensor_tensor(qkT[:, c, tsl], qkT[:, c, tsl], tmp2[:], ADD)

              def proj_v(sb_i, on_act=False):
                  psv = psum.tile([128, GF], F32, tag="sc", bufs=2)
                  for dc in range(8):
                      nc.tensor.matmul(
                          psv[:], xt_all[:, dc, sb_i * 128:(sb_i + 1) * 128],
                          wv_sb[:, dc, :], start=(dc == 0), stop=(dc == 7))
                  if on_act:
                      nc.scalar.copy(vt[:, sb_i, :, 0:DK],
                                     psv[:].rearrange("p (h d) -> p h d", h=GH))
                  else:
                      nc.vector.tensor_copy(
                          vt[:, sb_i, :, 0:DK],
                          psv[:].rearrange("p (h d) -> p h d", h=GH))

              # Deferred-PE-work queue: projection chunks for tile t+1,
              # transposes of the previous pair, and the previous qt's output
              # projection all get pumped into the attention kb-loop so the
              # (in-order) PE never sits behind a dependency-stalled
              # instruction for long.
              from collections import deque
              fill_q = deque()   # prompt PE work (projections, transposes)
              late_q = deque()   # output projections, deferred to late qts
                                 # where attention has an ACT-vs-PE deficit

              def pump(n=1, late_ok=False):
                  for _ in range(n):
                      if fill_q:
                          fill_q.popleft()()
                      elif late_ok and late_q:
                          late_q.popleft()()

              def transpose_unit(gq, pair, att_sb, qb):
                  def run():
                      tps = psum.tile([128, 128], BF16, tag="sc", bufs=2,
                                      name=f"tps{gq}_{pair}_{_it}")
                      nc.tensor.transpose(tps[:], att_sb[:, qb, :], ident_sb[:])
                      nc.vector.tensor_copy(
                          attnT[pair][:, gq * 128:(gq + 1) * 128], tps[:])
                  return run

              def oproj_unit(qb, split_copy=False):
                  # output projection for one 128-q-block
                  def run():
                      qsl = slice(qb * 128, (qb + 1) * 128)
                      osb = opool.tile([128, D], BF16, tag="osb",
                                       name=f"osb{qb}_{_it}")
                      for nh in range(2):
                          nsl = slice(nh * 512, (nh + 1) * 512)
                          pso = psum.tile([128, 512], F32, tag="sc", bufs=2)
                          nc.tensor.matmul(pso[:], attnT[0][:, qsl],
                                           wo_sb[:, 0, nsl], start=True, stop=False)
                          nc.tensor.matmul(pso[:], attnT[1][:, qsl],
                                           wo_sb[:, 1, nsl], start=False, stop=True)
                          if split_copy and nh == 1:
                              nc.scalar.copy(osb[:, nsl], pso[:])
                          else:
                              nc.vector.tensor_copy(osb[:, nsl], pso[:])
                      nc.sync.dma_start(out[qsl, :], osb[:])
                  return run

              # ---- attention (PV-flipped), deferred work interleaved ----
              # PV: attn[q, dk] = probsT.T @ [V|1] per 128-q-block: N=65 moving
              # rows instead of N=512, fully using the 128-wide K (k-positions)
              # and M (q) dims of the PE array. Softmax sums land in column 64
              # as per-partition scalars -> normalization via Pool broadcast
              # multiply, then a PE transpose restores [f, q] layout for the
              # output projection.
              attnT = [bpool.tile([128, S], BF16, tag=f"attnT{p}",
                                  name=f"attnT{p}_{_it}") for p in range(2)]

              # tile t=0: pair-0's needs (Q01, K01, V) up front; Q23/K23 queued.
              # fast=True / on_act=True shift prologue elementwise work onto
              # the idle ACT engine to shorten the first-attention latency.
              proj_qk(0, 0, fast=True)
              proj_qk(0, 2, fast=True)
              for s in range(4):
                  proj_v(s, on_act=True)
              fill_q.append(lambda: proj_qk(0, 1))
              fill_q.append(lambda: proj_qk(0, 3))

              state = {"att_next": None, "seq": 0}
              for qt in range(SB):
                  if qt + 1 < SB:
                      t = qt + 1
                      for c in range(4):
                          fill_q.append(lambda t=t, c=c: proj_qk(t, c))
                      for s in range(4 * t, 4 * t + 4):
                          fill_q.append(lambda s=s: proj_v(s))
                  nkb = 4 * qt + 4
                  nsteps = 2 * nkb
                  step = 0
                  for pair in range(2):
                      qs, ks = pair, 2 + pair
                      # Interleaved accumulation GROUPS in one PSUM bank are
                      # broken on HW (start=True wipes the bank), so banks are
                      # zeroed once (memset) and accumulated with start=False
                      # throughout. The memset for this pair was issued inside
                      # the PREVIOUS pair's normalize (right after that pair's
                      # last read of the same buffer) to shorten the handoff.
                      if state["att_next"] is None:
                          attps = [psum.tile([128, 4, DK + 1], F32, tag="att",
                                             bufs=2, name=f"attps{h}_{qt}_{pair}_{_it}")
                                   for h in range(2)]
                          for h in range(2):
                              nc.vector.memset(attps[h][:], 0.0)
                      else:
                          attps = state["att_next"]
                      state["att_next"] = None

                      def pv(kb):
                          for qb in range(max(kb - 4 * qt, 0), 4):
                              gq = 4 * qt + qb
                              for h in range(2):
                                  nc.tensor.matmul(
                                      attps[h][:, qb, :],
                                      pab_ring[kb % 4][:, h, qb * 128:(qb + 1) * 128],
                                      vt[:, kb, 2 * pair + h, :],
                                      start=False, stop=(kb == gq),
                                      skip_group_check=True)

                      att_sb = apool.tile([128, 4, 128], BF16, tag="attsb")
                      rsum = wpool.tile([128, 4, 2], F32, tag="rsum")
                      last = qt == SB - 1 and pair == 1

                      def normalize(qb):
                          # DVE reciprocal of the col-64 sums + DVE broadcast
                          # multiply (gpsimd cannot read PSUM). Per-q-block
                          # only for the very last pair (to pipeline the
                          # tail); batched per-pair otherwise to keep the DVE
                          # instruction count down. Transposes go to the FRONT
                          # of the queue: they are small, release the shared
                          # "sc" PSUM ring fast, and feed the output
                          # projection.
                          if not last:
                              if qb < 3:
                                  return
                              qsl3, nq = slice(0, 4), 4
                              # allocate + zero the NEXT pair's PSUM buffers
                              # h-by-h, right after this pair's last read of
                              # the same buffer, so h0 is ready before h1's
                              # normalize even runs
                              nxt = [psum.tile([128, 4, DK + 1], F32,
                                               tag="att", bufs=2,
                                               name=f"attps{h}_nx{state['seq']}_{_it}")
                                     for h in range(2)]
                              state["seq"] += 1
                              state["att_next"] = nxt
                          else:
                              qsl3, nq = slice(qb, qb + 1), 1
                              nxt = None
                          for h in range(2):
                              nc.vector.reciprocal(rsum[:, qsl3, h:h + 1],
                                                   attps[h][:, qsl3, DK:DK + 1])
                              nc.vector.tensor_tensor(
                                  att_sb[:, qsl3, h * 64:(h + 1) * 64],
                                  attps[h][:, qsl3, 0:DK],
                                  rsum[:, qsl3, h:h + 1].to_broadcast(
                                      [128, nq, DK]), MULT)
                              if nxt is not None:
                                  nc.vector.memset(nxt[h][:], 0.0)
                          if last:
                              # last qt: output projection chases each q-block
                              fill_q.appendleft(
                                  oproj_unit(4 * qt + qb, split_copy=True))
                              fill_q.appendleft(
                                  transpose_unit(4 * qt + qb, pair, att_sb, qb))
                          else:
                              for b in range(3, -1, -1):
                                  fill_q.appendleft(
                                      transpose_unit(4 * qt + b, pair, att_sb, b))

                      pab_ring = {}
                      for kb in range(nkb):
                          lam = max(kb - 4 * qt, 0) * 128
                          qsl = slice(qt * 512 + lam, (qt + 1) * 512)
                          ksl = slice(kb * 128, (kb + 1) * 128)
                          ss = psum.tile([128, 2, 512], F32, tag="sc2", bufs=2)
                          nc.tensor.matmul(ss[:, 0, lam:512], qkT[0:64, ks, ksl],
                                           qkT[0:64, qs, qsl], start=True, stop=True)
                          nc.tensor.matmul(ss[:, 1, lam:512], qkT[64:128, ks, ksl],
                                           qkT[64:128, qs, qsl], start=True, stop=True)
                          pab = ppool.tile([128, 2, 512], BF16, tag="probs")
                          pab_ring[kb % 4] = pab
                          nc.scalar.activation(pab[:, :, lam:512], ss[:, :, lam:512], AF.Exp, scale=0.125)
                          if kb >= 4 * qt:  # diagonal block: causal tri mask
                              dsl = slice(lam, lam + 128)
                              nc.gpsimd.tensor_tensor(
                                  pab[:, :, dsl], pab[:, :, dsl],
                                  tri_sb[:, None, :].to_broadcast([128, 2, 128]), MULT)
                          # drain queued PE work evenly across this qt's steps;
                          # late (output-projection) work backfills in the
                          # ACT-deficit qts
                          n = -(-len(fill_q) // (nsteps - step)) if fill_q else 0
                          if qt >= SB - 2:
                              n = max(n, 1)
                          pump(n, late_ok=(qt >= SB - 2))
                          step += 1
                          # software pipeline: PV for the previous kb runs
                          # after this kb's scores are already in flight
                          if kb > 0:
                              pv(kb - 1)
                              if kb - 1 >= 4 * qt:  # that region just stopped
                                  normalize(kb - 1 - 4 * qt)
                      pv(nkb - 1)
                      normalize(3)
                  if qt < SB - 1:
                      for qb in range(4 * qt, 4 * qt + 4):
                          late_q.append(oproj_unit(qb))
              while late_q:
                  late_q.popleft()()
              while fill_q:
                  fill_q.popleft()()

    nc.compile()
    return nc


def _host_tables(token_positions):
    pos = np.asarray(token_positions, dtype=np.float32)  # [S]
    half = DK // 2
    freq = THETA ** (-np.arange(0, DK, 2, dtype=np.float32) / DK)  # [32]
    # per-partition tables on [dk(128 = 2 heads of 64), s]
    f64 = np.repeat(freq, 2)          # [64] freq per feature index
    ang64 = pos[None, :] * f64[:, None]  # [64, S]
    cos64 = np.cos(ang64)
    sin64 = np.sin(ang64)
    sign = np.where(np.arange(DK) % 2 == 0, 1.0, -1.0).astype(np.float32)  # +s even, -s odd
    sins64 = sin64 * sign[:, None]
    from ml_dtypes import bfloat16 as bf16
    cosf = np.concatenate([cos64, cos64], axis=0).astype(bf16)   # [128, S]
    sins = np.concatenate([sins64, sins64], axis=0).astype(bf16)  # [128, S]
    return cosf, sins


def kernel(x, Wq, Wk, Wv, Wo, token_positions):
    from ml_dtypes import bfloat16 as bf16
    x = np.asarray(x, dtype=np.float32)
    Wq = np.asarray(Wq, dtype=np.float32)
    Wk = np.asarray(Wk, dtype=np.float32)
    Wv = np.asarray(Wv, dtype=np.float32)
    Wo = np.asarray(Wo, dtype=np.float32)

    if "nc" not in _CACHED:
        _CACHED["nc"] = _build_nc(iters=int(os.environ.get("BENCH_ITERS", "1")))
    nc = _CACHED["nc"]

    cosf, sins = _host_tables(token_positions)
    tri = np.triu(np.ones((128, 128), dtype=bf16))  # tri[k, j] = 1 if j >= k
    ident = np.eye(128, dtype=bf16)
    onesc = np.ones((128, 1), dtype=np.float32)

    xT = [np.ascontiguousarray(x[b].T).astype(bf16) for b in range(B)]  # [D, S]
    in_maps = []
    for c in range(NCORES):
        b, g = c // GROUPS, c % GROUPS
        R = slice(g * GF, (g + 1) * GF)
        wqkT = np.ascontiguousarray(
            np.concatenate([Wq[R].T, Wk[R].T], axis=1)).astype(bf16)  # [D, 512]
        wvT = np.ascontiguousarray(Wv[R].T).astype(bf16)              # [D, 256]
        woT = np.ascontiguousarray(Wo[:, R].T).astype(bf16)           # [256, D]
        in_maps.append({
            "xT": xT[b], "wqkT": wqkT, "wvT": wvT, "woT": woT,
            "cosf": cosf, "sins": sins, "tri": tri, "ident": ident, "onesc": onesc,
            "cachebust": np.zeros((int(os.environ.get("BENCH_ITERS", "1")), KVER), dtype=np.float32),
        })

    try:
        res = run_bass_kernel_spmd(nc, in_maps, core_ids=list(range(NCORES)))
    except Exception:
        # transient NRT_EXEC_UNIT_UNRECOVERABLE flakes recover on retry
        import time as _time
        _time.sleep(2.0)
        res = run_bass_kernel_spmd(nc, in_maps, core_ids=list(range(NCORES)))
    _CACHED["last_results"] = res
    outs = [np.asarray(r["out"], dtype=np.float32) for r in res.results]  # [S, D]
    full = np.empty((B, S, D), dtype=np.float32)
    for b in range(B):
        full[b] = sum(outs[b * GROUPS + g] for g in range(GROUPS))
    return full



# revision 73
# speedup vs baseline: 1.2749x; 1.0229x over previous
"""Causal MHA with RoPE on 8 TRN2 NeuronCores.

Sharding: data-parallel over batch (2) x tensor-parallel over heads (4 groups
of 4 heads) = 8 cores. Core c handles batch c//4, head group c%4.
Each core computes its 4 heads' attention and a partial output projection
(Wo sharded row-wise); host sums the 4 partials per batch.

Per-core device algorithm (all matmuls in float32r = TF32, fp32 accumulate):
  - QK^T projection: qkT[dk, s] = (Wqk rows).T-contracted with xT (host-transposed x)
  - RoPE applied on [dk(partition), s] layout via cos/sin tables and a
    stream_shuffle partition pair-swap
  - scores^T[k, q] = K^T.T-free @ Q^T per head (K=64 contraction, two heads
    packed in row groups 0-1 / 2-3 of the PE array)
  - probsT = exp(scores/8) straight from PSUM (no max subtraction; scores are
    N(0,1)-scaled so exp never overflows), causal tri-mask on diagonal tiles
  - attnT_unnorm[dk, q] (+ row of sums via a ones column in [V|1]) = [V|1].T @ probsT
  - softmax normalization: sums row -> K=1 broadcast matmul -> reciprocal ->
    one elementwise multiply
  - partial out = attnT.T-contracted with WoT chunks, accumulated over the
    2 head pairs, DMA'd to DRAM
"""
import sys
import os

for _p in ("/opt/trn_rl_repo", "/root/.axon_site/_ro/trn_rl_repo"):
    if os.path.isdir(_p) and _p not in sys.path:
        sys.path.insert(0, _p)

import numpy as np

import concourse.mybir as mybir
import concourse.tile as tile
from concourse import bacc
from concourse.bass_utils import run_bass_kernel_spmd

F32 = mybir.dt.float32
F32R = mybir.dt.float32r
BF16 = mybir.dt.bfloat16
AF = mybir.ActivationFunctionType
MULT = mybir.AluOpType.mult
ADD = mybir.AluOpType.add
DIV = mybir.AluOpType.divide

B, S, D = 2, 2048, 1024
H, DK = 16, 64
THETA = 10000.0
NCORES = 8
GROUPS = 4          # head groups (tensor parallel)
GH = H // GROUPS    # heads per group = 4
GF = GH * DK        # features per group = 256
SWAP_MASK = [i ^ 1 for i in range(32)]
KVER = 16  # bump on any kernel change: busts the HLO-shape-keyed NEFF cache

_CACHED = {}


def _build_nc(iters=1):
    _iters = iters
    nc = bacc.Bacc("TRN2", target_bir_lowering=False, debug=False, num_devices=NCORES)
    xT = nc.dram_tensor("xT", [D, S], BF16, kind="ExternalInput").ap()
    wqkT = nc.dram_tensor("wqkT", [D, 2 * GF], BF16, kind="ExternalInput").ap()
    wvT = nc.dram_tensor("wvT", [D, GF], BF16, kind="ExternalInput").ap()
    woT = nc.dram_tensor("woT", [GF, D], BF16, kind="ExternalInput").ap()
    cosf = nc.dram_tensor("cosf", [128, S], BF16, kind="ExternalInput").ap()
    sins = nc.dram_tensor("sins", [128, S], BF16, kind="ExternalInput").ap()
    tri = nc.dram_tensor("tri", [128, 128], BF16, kind="ExternalInput").ap()
    ident = nc.dram_tensor("ident", [128, 128], BF16, kind="ExternalInput").ap()
    onesc = nc.dram_tensor("onesc", [128, 1], F32R, kind="ExternalInput").ap()
    # unused input whose shape encodes the kernel version: the neuron compile
    # cache keys on HLO structure only, so two kernels with identical I/O
    # shapes would otherwise collide.
    nc.dram_tensor("cachebust", [iters, KVER], F32, kind="ExternalInput")
    out = nc.dram_tensor("out", [S, D], BF16, kind="ExternalOutput").ap()

    SB = S // 512  # 4 q-tiles of 512
    KB = S // 128  # 16 k-blocks of 128

    with tile.TileContext(nc) as tc:
        with tc.tile_pool(name="const", bufs=1) as cpool, \
             tc.tile_pool(name="big", bufs=1) as bpool, \
             tc.tile_pool(name="work", bufs=2) as wpool, \
             tc.tile_pool(name="asb", bufs=3) as apool, \
             tc.tile_pool(name="probs", bufs=4) as ppool, \
             tc.tile_pool(name="obuf", bufs=3) as opool, \
             tc.tile_pool(name="psum", bufs=1, space="PSUM") as psum:

            # ---- loads, ordered by first use on the single HWDGE queue ----
            wqk_sb = cpool.tile([128, 8, 2 * GF], BF16, tag="wqk")
            wv_sb = cpool.tile([128, 8, GF], BF16, tag="wv")
            wo_sb = cpool.tile([128, 2, D], BF16, tag="wo")
            cos_sb = cpool.tile([128, S], BF16, tag="cos")
            sin_sb = cpool.tile([128, S], BF16, tag="sin")
            tri_sb = cpool.tile([128, 128], BF16, tag="tri")
            ident_sb = cpool.tile([128, 128], BF16, tag="ident")
            onesc_sb = cpool.tile([128, 1], F32R, tag="onesc")
            xt_all = cpool.tile([128, 8, S], BF16, tag="xt")

            def xt_load(tsl, nway=2):
                # x feature-chunks dc packed on partitions, batched DMAs
                w = 8 // nway
                for i in range(nway):
                    nc.sync.dma_start(
                        xt_all[:, i * w:(i + 1) * w, tsl],
                        xT[i * w * 128:(i + 1) * w * 128, tsl].rearrange(
                            "(dc p) s -> p dc s", p=128))

            def wqk_load(c):
                nc.sync.dma_start(
                    wqk_sb[:, :, c * 128:(c + 1) * 128],
                    wqkT[:, c * 128:(c + 1) * 128].rearrange(
                        "(dc p) n -> p dc n", p=128))

            wqk_load(0)
            xt_load(slice(0, 512), nway=4)
            nc.sync.dma_start(wv_sb[:], wvT.rearrange("(dc p) n -> p dc n", p=128))
            wqk_load(2)
            nc.sync.dma_start(onesc_sb[:], onesc)
            nc.sync.dma_start(cos_sb[:, 0:1024], cosf[:, 0:1024])
            nc.sync.dma_start(sin_sb[:, 0:1024], sins[:, 0:1024])
            wqk_load(1)
            wqk_load(3)
            nc.sync.dma_start(tri_sb[:], tri)
            xt_load(slice(512, 1024))
            nc.sync.dma_start(ident_sb[:], ident)
            nc.sync.dma_start(cos_sb[:, 1024:S], cosf[:, 1024:S])
            nc.sync.dma_start(sin_sb[:, 1024:S], sins[:, 1024:S])
            nc.sync.dma_start(wo_sb[:], woT.rearrange("(fc p) n -> p fc n", p=128))
            xt_load(slice(1024, S))

            warm = cpool.tile([1, 1], F32, tag="warm")
            nc.scalar.activation(warm[:], onesc_sb[0:1, 0:1], AF.Exp, scale=1.0)

            # ---- kernel body ----
            # Projections (phase 1) for x-tile t+1 are interleaved INTO the
            # attention kb-loop over qt=t: attention is ACT(exp)-bound, so the
            # PE fills its idle slots with the next tile's QK/V projections.
            for _it in range(iters):
              qkT = bpool.tile([128, 4, S], BF16, tag="qkT", name=f"qkT{_it}")
              vt = bpool.tile([128, KB, GH, DK + 1], BF16, tag="vt", name=f"vt{_it}")
              nc.vector.tensor_copy(
                  vt[:, :, :, DK:DK + 1],
                  onesc_sb[:, None, None, :].to_broadcast([128, KB, GH, 1]))

              def proj_qk_half(t, c, half, ps):
                  for dc in range(4 * half, 4 * half + 4):
                      nc.tensor.matmul(
                          ps[:], wqk_sb[:, dc, c * 128:(c + 1) * 128],
                          xt_all[:, dc, t * 512:(t + 1) * 512],
                          start=(dc == 0), stop=(dc == 7))

              def proj_qk(t, c, fast=False, half=None):
                  # QK projection chunk: 128 features (head pair c of Q|K),
                  # 512 seq positions, full D contraction; then RoPE.
                  ps = psum.tile([128, 512], F32, tag="sc", bufs=2)
                  proj_qk_half(t, c, 0, ps)
                  if half is not None:
                      # second half (+ RoPE) deferred as its own filler unit
                      half.append(lambda: proj_qk_rope(t, c, ps, fast))
                      return
                  proj_qk_rope(t, c, ps, fast)

              def proj_qk_rope(t, c, ps, fast=False):
                  proj_qk_half(t, c, 1, ps)
                  tsl = slice(t * 512, (t + 1) * 512)
                  # rope: qkT = ps*cos + swap(ps*sins), all on DVE (Pool is
                  # reserved for the latency-critical causal masks)
                  if fast:
                      # prologue chunks: pre-round ps to bf16 on the (idle)
                      # ACT engine so the DVE multiplies run in 2x mode
                      psb = wpool.tile([128, 512], BF16, tag="psb")
                      nc.scalar.copy(psb[:], ps[:])
                      src = psb
                  else:
                      src = ps
                  tmp = wpool.tile([128, 512], BF16, tag="ropetmp")
                  nc.vector.tensor_tensor(tmp[:], src[:], sin_sb[:, tsl], MULT)
                  tmp2 = wpool.tile([128, 512], BF16, tag="ropetmp2")
                  nc.vector.stream_shuffle(tmp2[:], tmp[:], SWAP_MASK)
                  nc.vector.tensor_tensor(qkT[:, c, tsl], src[:], cos_sb[:, tsl], MULT)
                  nc.vector.tensor_tensor(qkT[:, c, tsl], qkT[:, c, tsl], tmp2[:], ADD)

              def proj_v(sb_i, on_act=False):
                  psv = psum.tile([128, GF], F32, tag="sc", bufs=2)
                  for dc in range(8):
                      nc.tensor.matmul(
                          psv[:], xt_all[:, dc, sb_i * 128:(sb_i + 1) * 128],
                          wv_sb[:, dc, :], start=(dc == 0), stop=(dc == 7))
                  if on_act:
                      nc.scalar.copy(vt[:, sb_i, :, 0:DK],
                                     psv[:].rearrange("p (h d) -> p h d", h=GH))
                  else:
                      nc.vector.tensor_copy(
                          vt[:, sb_i, :, 0:DK],
                          psv[:].rearrange("p (h d) -> p h d", h=GH))

              # Deferred-PE-work queue: projection chunks for tile t+1,
              # transposes of the previous pair, and the previous qt's output
              # projection all get pumped into the attention kb-loop so the
              # (in-order) PE never sits behind a dependency-stalled
              # instruction for long.
              from collections import deque
              fill_q = deque()   # prompt PE work (projections, transposes)
              late_q = deque()   # output projections, deferred to late qts
                                 # where attention has an ACT-vs-PE deficit

              def pump(n=1, late_ok=False):
                  for _ in range(n):
                      if fill_q:
                          fill_q.popleft()()
                      elif late_ok and late_q:
                          late_q.popleft()()

              def transpose_unit(gq, pair, att_sb, qb):
                  def run():
                      tps = psum.tile([128, 128], BF16, tag="sc", bufs=2,
                                      name=f"tps{gq}_{pair}_{_it}")
                      nc.tensor.transpose(tps[:], att_sb[:, qb, :], ident_sb[:])
                      nc.vector.tensor_copy(
                          attnT[pair][:, gq * 128:(gq + 1) * 128], tps[:])
                  return run

              def oproj_unit(qb, split_copy=False):
                  # output projection for one 128-q-block
                  def run():
                      qsl = slice(qb * 128, (qb + 1) * 128)
                      osb = opool.tile([128, D], BF16, tag="osb",
                                       name=f"osb{qb}_{_it}")
                      for nh in range(2):
                          nsl = slice(nh * 512, (nh + 1) * 512)
                          pso = psum.tile([128, 512], F32, tag="sc", bufs=2)
                          nc.tensor.matmul(pso[:], attnT[0][:, qsl],
                                           wo_sb[:, 0, nsl], start=True, stop=False)
                          nc.tensor.matmul(pso[:], attnT[1][:, qsl],
                                           wo_sb[:, 1, nsl], start=False, stop=True)
                          if split_copy and nh == 1:
                              nc.scalar.copy(osb[:, nsl], pso[:])
                          else:
                              nc.vector.tensor_copy(osb[:, nsl], pso[:])
                      nc.sync.dma_start(out[qsl, :], osb[:])
                  return run

              # ---- attention (PV-flipped), deferred work interleaved ----
              # PV: attn[q, dk] = probsT.T @ [V|1] per 128-q-block: N=65 moving
              # rows instead of N=512, fully using the 128-wide K (k-positions)
              # and M (q) dims of the PE array. Softmax sums land in column 64
              # as per-partition scalars -> normalization via Pool broadcast
              # multiply, then a PE transpose restores [f, q] layout for the
              # output projection.
              attnT = [bpool.tile([128, S], BF16, tag=f"attnT{p}",
                                  name=f"attnT{p}_{_it}") for p in range(2)]

              # tile t=0: pair-0's needs (Q01, K01, V) up front; Q23/K23 queued.
              # fast=True / on_act=True shift prologue elementwise work onto
              # the idle ACT engine to shorten the first-attention latency.
              proj_qk(0, 0, fast=True)
              proj_qk(0, 2, fast=True)
              for s in range(4):
                  proj_v(s, on_act=True)
              fill_q.append(lambda: proj_qk(0, 1, fast=True))
              fill_q.append(lambda: proj_qk(0, 3, fast=True))

              state = {"att_next": None, "seq": 0}
              for qt in range(SB):
                  if qt + 1 < SB:
                      t = qt + 1
                      for c in range(4):
                          fill_q.append(lambda t=t, c=c: proj_qk(t, c, fast=True))
                      for s in range(4 * t, 4 * t + 4):
                          fill_q.append(lambda s=s: proj_v(s))
                  nkb = 4 * qt + 4
                  nsteps = 2 * nkb
                  step = 0
                  for pair in range(2):
                      qs, ks = pair, 2 + pair
                      # Interleaved accumulation GROUPS in one PSUM bank are
                      # broken on HW: start=True zeroes the WHOLE bank (HW
                      # verified). Exploit that: the first PV write of each
                      # h-bank (kb=0, qb=0) runs with start=True to zero the
                      # bank, everything else accumulates with start=False.
                      attps = [psum.tile([128, 4, DK + 1], F32, tag="att",
                                         bufs=2, name=f"attps{h}_{qt}_{pair}_{_it}")
                               for h in range(2)]

                      def pv(kb):
                          for qb in range(max(kb - 4 * qt, 0), 4):
                              gq = 4 * qt + qb
                              for h in range(2):
                                  nc.tensor.matmul(
                                      attps[h][:, qb, :],
                                      pab_ring[kb % 4][:, h, qb * 128:(qb + 1) * 128],
                                      vt[:, kb, 2 * pair + h, :],
                                      start=(kb == 0 and qb == 0),
                                      stop=(kb == gq),
                                      skip_group_check=True)

                      att_sb = apool.tile([128, 4, 128], BF16, tag="attsb")
                      rsum = wpool.tile([128, 4, 2], F32, tag="rsum")
                      last = qt == SB - 1 and pair == 1

                      def normalize(qb):
                          # DVE reciprocal of the col-64 sums + DVE broadcast
                          # multiply (gpsimd cannot read PSUM). Per-q-block
                          # only for the very last pair (to pipeline the
                          # tail); batched per-pair otherwise to keep the DVE
                          # instruction count down. Transposes go to the FRONT
                          # of the queue: they are small, release the shared
                          # "sc" PSUM ring fast, and feed the output
                          # projection.
                          if not last:
                              if qb < 3:
                                  return
                              qsl3, nq = slice(0, 4), 4
                          else:
                              qsl3, nq = slice(qb, qb + 1), 1
                          for h in range(2):
                              nc.vector.reciprocal(rsum[:, qsl3, h:h + 1],
                                                   attps[h][:, qsl3, DK:DK + 1])
                              nc.vector.tensor_tensor(
                                  att_sb[:, qsl3, h * 64:(h + 1) * 64],
                                  attps[h][:, qsl3, 0:DK],
                                  rsum[:, qsl3, h:h + 1].to_broadcast(
                                      [128, nq, DK]), MULT)
                          if last:
                              # last qt: output projection chases each q-block
                              fill_q.appendleft(
                                  oproj_unit(4 * qt + qb, split_copy=True))
                              fill_q.appendleft(
                                  transpose_unit(4 * qt + qb, pair, att_sb, qb))
                          else:
                              for b in range(3, -1, -1):
                                  fill_q.appendleft(
                                      transpose_unit(4 * qt + b, pair, att_sb, b))

                      pab_ring = {}
                      for kb in range(nkb):
                          lam = max(kb - 4 * qt, 0) * 128
                          qsl = slice(qt * 512 + lam, (qt + 1) * 512)
                          ksl = slice(kb * 128, (kb + 1) * 128)
                          ss = psum.tile([128, 2, 512], F32, tag="sc2", bufs=2)
                          nc.tensor.matmul(ss[:, 0, lam:512], qkT[0:64, ks, ksl],
                                           qkT[0:64, qs, qsl], start=True, stop=True)
                          nc.tensor.matmul(ss[:, 1, lam:512], qkT[64:128, ks, ksl],
                                           qkT[64:128, qs, qsl], start=True, stop=True)
                          pab = ppool.tile([128, 2, 512], BF16, tag="probs")
                          pab_ring[kb % 4] = pab
                          nc.scalar.activation(pab[:, :, lam:512], ss[:, :, lam:512], AF.Exp, scale=0.125)
                          if kb >= 4 * qt:  # diagonal block: causal tri mask
                              dsl = slice(lam, lam + 128)
                              nc.gpsimd.tensor_tensor(
                                  pab[:, :, dsl], pab[:, :, dsl],
                                  tri_sb[:, None, :].to_broadcast([128, 2, 128]), MULT)
                          # drain queued PE work evenly across this qt's steps;
                          # late (output-projection) work backfills in the
                          # ACT-deficit qts
                          n = -(-len(fill_q) // (nsteps - step)) if fill_q else 0
                          if qt >= SB - 2:
                              n = max(n, 1)
                          pump(n, late_ok=(qt >= SB - 2))
                          step += 1
                          # software pipeline: PV for the previous kb runs
                          # after this kb's scores are already in flight
                          if kb > 0:
                              pv(kb - 1)
                              if kb - 1 >= 4 * qt:  # that region just stopped
                                  normalize(kb - 1 - 4 * qt)
                      pv(nkb - 1)
                      normalize(3)
                  if qt < SB - 1:
                      for qb in range(4 * qt, 4 * qt + 4):
                          late_q.append(oproj_unit(qb))
              while late_q:
                  late_q.popleft()()
              while fill_q:
                  fill_q.popleft()()

    nc.compile()
    return nc


def _host_tables(token_positions):
    pos = np.asarray(token_positions, dtype=np.float32)  # [S]
    half = DK // 2
    freq = THETA ** (-np.arange(0, DK, 2, dtype=np.float32) / DK)  # [32]
    # per-partition tables on [dk(128 = 2 heads of 64), s]
    f64 = np.repeat(freq, 2)          # [64] freq per feature index
    ang64 = pos[None, :] * f64[:, None]  # [64, S]
    cos64 = np.cos(ang64)
    sin64 = np.sin(ang64)
    sign = np.where(np.arange(DK) % 2 == 0, 1.0, -1.0).astype(np.float32)  # +s even, -s odd
    sins64 = sin64 * sign[:, None]
    from ml_dtypes import bfloat16 as bf16
    cosf = np.concatenate([cos64, cos64], axis=0).astype(bf16)   # [128, S]
    sins = np.concatenate([sins64, sins64], axis=0).astype(bf16)  # [128, S]
    return cosf, sins


def kernel(x, Wq, Wk, Wv, Wo, token_positions):
    from ml_dtypes import bfloat16 as bf16
    x = np.asarray(x, dtype=np.float32)
    Wq = np.asarray(Wq, dtype=np.float32)
    Wk = np.asarray(Wk, dtype=np.float32)
    Wv = np.asarray(Wv, dtype=np.float32)
    Wo = np.asarray(Wo, dtype=np.float32)

    if "nc" not in _CACHED:
        _CACHED["nc"] = _build_nc(iters=int(os.environ.get("BENCH_ITERS", "1")))
    nc = _CACHED["nc"]

    cosf, sins = _host_tables(token_positions)
    tri = np.triu(np.ones((128, 128), dtype=bf16))  # tri[k, j] = 1 if j >= k
    ident = np.eye(128, dtype=bf16)
    onesc = np.ones((128, 1), dtype=np.float32)

    xT = [np.ascontiguousarray(x[b].T).astype(bf16) for b in range(B)]  # [D, S]
    in_maps = []
    for c in range(NCORES):
        b, g = c // GROUPS, c % GROUPS
        R = slice(g * GF, (g + 1) * GF)
        wqkT = np.ascontiguousarray(
            np.concatenate([Wq[R].T, Wk[R].T], axis=1)).astype(bf16)  # [D, 512]
        wvT = np.ascontiguousarray(Wv[R].T).astype(bf16)              # [D, 256]
        woT = np.ascontiguousarray(Wo[:, R].T).astype(bf16)           # [256, D]
        in_maps.append({
            "xT": xT[b], "wqkT": wqkT, "wvT": wvT, "woT": woT,
            "cosf": cosf, "sins": sins, "tri": tri, "ident": ident, "onesc": onesc,
            "cachebust": np.zeros((int(os.environ.get("BENCH_ITERS", "1")), KVER), dtype=np.float32),
        })

    try:
        res = run_bass_kernel_spmd(nc, in_maps, core_ids=list(range(NCORES)))
    except Exception:
        # transient NRT_EXEC_UNIT_UNRECOVERABLE flakes recover on retry
        import time as _time
        _time.sleep(2.0)
        res = run_bass_kernel_spmd(nc, in_maps, core_ids=list(range(NCORES)))
    _CACHED["last_results"] = res
    outs = [np.asarray(r["out"], dtype=np.float32) for r in res.results]  # [S, D]
    full = np.empty((B, S, D), dtype=np.float32)
    for b in range(B):
        full[b] = sum(outs[b * GROUPS + g] for g in range(GROUPS))
    return full



# revision 78
# speedup vs baseline: 1.3302x; 1.0433x over previous
"""Causal MHA with RoPE on 8 TRN2 NeuronCores.

Sharding: data-parallel over batch (2) x tensor-parallel over heads (4 groups
of 4 heads) = 8 cores. Core c handles batch c//4, head group c%4.
Each core computes its 4 heads' attention and a partial output projection
(Wo sharded row-wise); host sums the 4 partials per batch.

Per-core device algorithm (all matmuls in float32r = TF32, fp32 accumulate):
  - QK^T projection: qkT[dk, s] = (Wqk rows).T-contracted with xT (host-transposed x)
  - RoPE applied on [dk(partition), s] layout via cos/sin tables and a
    stream_shuffle partition pair-swap
  - scores^T[k, q] = K^T.T-free @ Q^T per head (K=64 contraction, two heads
    packed in row groups 0-1 / 2-3 of the PE array)
  - probsT = exp(scores/8) straight from PSUM (no max subtraction; scores are
    N(0,1)-scaled so exp never overflows), causal tri-mask on diagonal tiles
  - attnT_unnorm[dk, q] (+ row of sums via a ones column in [V|1]) = [V|1].T @ probsT
  - softmax normalization: sums row -> K=1 broadcast matmul -> reciprocal ->
    one elementwise multiply
  - partial out = attnT.T-contracted with WoT chunks, accumulated over the
    2 head pairs, DMA'd to DRAM
"""
import sys
import os

for _p in ("/opt/trn_rl_repo", "/root/.axon_site/_ro/trn_rl_repo"):
    if os.path.isdir(_p) and _p not in sys.path:
        sys.path.insert(0, _p)

import numpy as np

import concourse.mybir as mybir
import concourse.tile as tile
from concourse import bacc
from concourse.bass_utils import run_bass_kernel_spmd

F32 = mybir.dt.float32
F32R = mybir.dt.float32r
BF16 = mybir.dt.bfloat16
AF = mybir.ActivationFunctionType
MULT = mybir.AluOpType.mult
ADD = mybir.AluOpType.add
DIV = mybir.AluOpType.divide

B, S, D = 2, 2048, 1024
H, DK = 16, 64
THETA = 10000.0
NCORES = 8
GROUPS = 4          # head groups (tensor parallel)
GH = H // GROUPS    # heads per group = 4
GF = GH * DK        # features per group = 256
SWAP_MASK = [i ^ 1 for i in range(32)]
KVER = 16  # bump on any kernel change: busts the HLO-shape-keyed NEFF cache

_CACHED = {}


def _build_nc(iters=1):
    _iters = iters
    nc = bacc.Bacc("TRN2", target_bir_lowering=False, debug=False, num_devices=NCORES)
    xT = nc.dram_tensor("xT", [D, S], BF16, kind="ExternalInput").ap()
    wqkT = nc.dram_tensor("wqkT", [D, 2 * GF], BF16, kind="ExternalInput").ap()
    wvT = nc.dram_tensor("wvT", [D, GF], BF16, kind="ExternalInput").ap()
    woT = nc.dram_tensor("woT", [GF, D], BF16, kind="ExternalInput").ap()
    cosf = nc.dram_tensor("cosf", [128, S], BF16, kind="ExternalInput").ap()
    sins = nc.dram_tensor("sins", [128, S], BF16, kind="ExternalInput").ap()
    tri = nc.dram_tensor("tri", [128, 128], BF16, kind="ExternalInput").ap()
    ident = nc.dram_tensor("ident", [128, 128], BF16, kind="ExternalInput").ap()
    onesc = nc.dram_tensor("onesc", [128, 1], F32R, kind="ExternalInput").ap()
    # unused input whose shape encodes the kernel version: the neuron compile
    # cache keys on HLO structure only, so two kernels with identical I/O
    # shapes would otherwise collide.
    nc.dram_tensor("cachebust", [iters, KVER], F32, kind="ExternalInput")
    out = nc.dram_tensor("out", [S, D], BF16, kind="ExternalOutput").ap()

    SB = S // 512  # 4 q-tiles of 512
    KB = S // 128  # 16 k-blocks of 128

    with tile.TileContext(nc) as tc:
        with tc.tile_pool(name="const", bufs=1) as cpool, \
             tc.tile_pool(name="big", bufs=1) as bpool, \
             tc.tile_pool(name="work", bufs=2) as wpool, \
             tc.tile_pool(name="asb", bufs=3) as apool, \
             tc.tile_pool(name="probs", bufs=4) as ppool, \
             tc.tile_pool(name="obuf", bufs=3) as opool, \
             tc.tile_pool(name="psum", bufs=1, space="PSUM") as psum:

            # ---- loads, ordered by first use on the single HWDGE queue ----
            wqk_sb = cpool.tile([128, 8, 2 * GF], BF16, tag="wqk")
            wv_sb = cpool.tile([128, 8, GF], BF16, tag="wv")
            wo_sb = cpool.tile([128, 2, D], BF16, tag="wo")
            cos_sb = cpool.tile([128, S], BF16, tag="cos")
            sin_sb = cpool.tile([128, S], BF16, tag="sin")
            tri_sb = cpool.tile([128, 128], BF16, tag="tri")
            ident_sb = cpool.tile([128, 128], BF16, tag="ident")
            onesc_sb = cpool.tile([128, 1], F32R, tag="onesc")
            xt_all = cpool.tile([128, 8, S], BF16, tag="xt")

            def xt_load(tsl, nway=2):
                # x feature-chunks dc packed on partitions, batched DMAs
                w = 8 // nway
                for i in range(nway):
                    nc.sync.dma_start(
                        xt_all[:, i * w:(i + 1) * w, tsl],
                        xT[i * w * 128:(i + 1) * w * 128, tsl].rearrange(
                            "(dc p) s -> p dc s", p=128))

            def wqk_load(c):
                nc.sync.dma_start(
                    wqk_sb[:, :, c * 128:(c + 1) * 128],
                    wqkT[:, c * 128:(c + 1) * 128].rearrange(
                        "(dc p) n -> p dc n", p=128))

            wqk_load(0)
            xt_load(slice(0, 512), nway=4)
            nc.sync.dma_start(wv_sb[:], wvT.rearrange("(dc p) n -> p dc n", p=128))
            wqk_load(2)
            nc.sync.dma_start(onesc_sb[:], onesc)
            nc.sync.dma_start(cos_sb[:, 0:1024], cosf[:, 0:1024])
            nc.sync.dma_start(sin_sb[:, 0:1024], sins[:, 0:1024])
            wqk_load(1)
            wqk_load(3)
            nc.sync.dma_start(tri_sb[:], tri)
            xt_load(slice(512, 1024))
            nc.sync.dma_start(ident_sb[:], ident)
            nc.sync.dma_start(cos_sb[:, 1024:S], cosf[:, 1024:S])
            nc.sync.dma_start(sin_sb[:, 1024:S], sins[:, 1024:S])
            nc.sync.dma_start(wo_sb[:], woT.rearrange("(fc p) n -> p fc n", p=128))
            xt_load(slice(1024, S))

            warm = cpool.tile([1, 1], F32, tag="warm")
            nc.scalar.activation(warm[:], onesc_sb[0:1, 0:1], AF.Exp, scale=1.0)

            # ---- kernel body ----
            # Projections (phase 1) for x-tile t+1 are interleaved INTO the
            # attention kb-loop over qt=t: attention is ACT(exp)-bound, so the
            # PE fills its idle slots with the next tile's QK/V projections.
            for _it in range(iters):
              qkT = bpool.tile([128, 4, S], BF16, tag="qkT", name=f"qkT{_it}")
              vt = bpool.tile([128, KB, GH, DK + 1], BF16, tag="vt", name=f"vt{_it}")
              nc.vector.tensor_copy(
                  vt[:, :, :, DK:DK + 1],
                  onesc_sb[:, None, None, :].to_broadcast([128, KB, GH, 1]))

              def proj_qk_half(t, c, half, ps):
                  for dc in range(4 * half, 4 * half + 4):
                      nc.tensor.matmul(
                          ps[:], wqk_sb[:, dc, c * 128:(c + 1) * 128],
                          xt_all[:, dc, t * 512:(t + 1) * 512],
                          start=(dc == 0), stop=(dc == 7))

              def proj_qk(t, c, fast=False, half=None):
                  # QK projection chunk: 128 features (head pair c of Q|K),
                  # 512 seq positions, full D contraction; then RoPE.
                  ps = psum.tile([128, 512], F32, tag="sc", bufs=2)
                  proj_qk_half(t, c, 0, ps)
                  if half is not None:
                      # second half (+ RoPE) deferred as the next filler unit
                      half.appendleft(lambda: proj_qk_rope(t, c, ps, fast))
                      return
                  proj_qk_rope(t, c, ps, fast)

              def proj_qk_rope(t, c, ps, fast=False):
                  proj_qk_half(t, c, 1, ps)
                  tsl = slice(t * 512, (t + 1) * 512)
                  # rope: qkT = ps*cos + swap(ps*sins), all on DVE (Pool is
                  # reserved for the latency-critical causal masks)
                  if fast:
                      # prologue chunks: pre-round ps to bf16 on the (idle)
                      # ACT engine so the DVE multiplies run in 2x mode
                      psb = wpool.tile([128, 512], BF16, tag="psb")
                      nc.scalar.copy(psb[:], ps[:])
                      src = psb
                  else:
                      src = ps
                  tmp = wpool.tile([128, 512], BF16, tag="ropetmp")
                  nc.vector.tensor_tensor(tmp[:], src[:], sin_sb[:, tsl], MULT)
                  tmp2 = wpool.tile([128, 512], BF16, tag="ropetmp2")
                  nc.vector.stream_shuffle(tmp2[:], tmp[:], SWAP_MASK)
                  nc.vector.tensor_tensor(qkT[:, c, tsl], src[:], cos_sb[:, tsl], MULT)
                  nc.vector.tensor_tensor(qkT[:, c, tsl], qkT[:, c, tsl], tmp2[:], ADD)

              def proj_v(sb_i, on_act=False):
                  psv = psum.tile([128, GF], F32, tag="sc", bufs=2)
                  for dc in range(8):
                      nc.tensor.matmul(
                          psv[:], xt_all[:, dc, sb_i * 128:(sb_i + 1) * 128],
                          wv_sb[:, dc, :], start=(dc == 0), stop=(dc == 7))
                  if on_act:
                      nc.scalar.copy(vt[:, sb_i, :, 0:DK],
                                     psv[:].rearrange("p (h d) -> p h d", h=GH))
                  else:
                      nc.vector.tensor_copy(
                          vt[:, sb_i, :, 0:DK],
                          psv[:].rearrange("p (h d) -> p h d", h=GH))

              # Deferred-PE-work queue: projection chunks for tile t+1,
              # transposes of the previous pair, and the previous qt's output
              # projection all get pumped into the attention kb-loop so the
              # (in-order) PE never sits behind a dependency-stalled
              # instruction for long.
              from collections import deque
              fill_q = deque()   # prompt PE work (projections, transposes)
              late_q = deque()   # output projections, deferred to late qts
                                 # where attention has an ACT-vs-PE deficit

              def pump(n=1, late_ok=False):
                  for _ in range(n):
                      if fill_q:
                          fill_q.popleft()()
                      elif late_ok and late_q:
                          late_q.popleft()()

              def transpose_unit(gq, pair, att_sb, qb):
                  def run():
                      tps = psum.tile([128, 128], BF16, tag="sc", bufs=2,
                                      name=f"tps{gq}_{pair}_{_it}")
                      nc.tensor.transpose(tps[:], att_sb[:, qb, :], ident_sb[:])
                      nc.vector.tensor_copy(
                          attnT[pair][:, gq * 128:(gq + 1) * 128], tps[:])
                  return run

              def oproj_units(qb, split_copy=False):
                  # output projection for one 128-q-block, as two filler units
                  # (one per 512-wide n-half; the second issues the DMA)
                  st = {}

                  def run_nh(nh):
                      qsl = slice(qb * 128, (qb + 1) * 128)
                      if nh == 0:
                          st["osb"] = opool.tile([128, D], BF16, tag="osb",
                                                 name=f"osb{qb}_{_it}")
                      osb = st["osb"]
                      nsl = slice(nh * 512, (nh + 1) * 512)
                      pso = psum.tile([128, 512], F32, tag="sc", bufs=2)
                      nc.tensor.matmul(pso[:], attnT[0][:, qsl],
                                       wo_sb[:, 0, nsl], start=True, stop=False)
                      nc.tensor.matmul(pso[:], attnT[1][:, qsl],
                                       wo_sb[:, 1, nsl], start=False, stop=True)
                      if split_copy and nh == 1:
                          nc.scalar.copy(osb[:, nsl], pso[:])
                      else:
                          nc.vector.tensor_copy(osb[:, nsl], pso[:])
                      if nh == 1:
                          nc.sync.dma_start(out[qsl, :], osb[:])

                  return [lambda: run_nh(0), lambda: run_nh(1)]

              # ---- attention (PV-flipped), deferred work interleaved ----
              # PV: attn[q, dk] = probsT.T @ [V|1] per 128-q-block: N=65 moving
              # rows instead of N=512, fully using the 128-wide K (k-positions)
              # and M (q) dims of the PE array. Softmax sums land in column 64
              # as per-partition scalars -> normalization via Pool broadcast
              # multiply, then a PE transpose restores [f, q] layout for the
              # output projection.
              attnT = [bpool.tile([128, S], BF16, tag=f"attnT{p}",
                                  name=f"attnT{p}_{_it}") for p in range(2)]

              # tile t=0: pair-0's needs (Q01, K01, V) up front; Q23/K23 queued.
              # fast=True / on_act=True shift prologue elementwise work onto
              # the idle ACT engine to shorten the first-attention latency.
              proj_qk(0, 0, fast=True)
              proj_qk(0, 2, fast=True)
              for s in range(4):
                  proj_v(s, on_act=True)
              fill_q.append(lambda: proj_qk(0, 1, fast=True))
              fill_q.append(lambda: proj_qk(0, 3, fast=True))

              state = {"att_next": None, "seq": 0}
              for qt in range(SB):
                  if qt + 1 < SB:
                      t = qt + 1
                      for c in range(4):
                          fill_q.append(lambda t=t, c=c: proj_qk(t, c, fast=True, half=fill_q))
                      for s in range(4 * t, 4 * t + 4):
                          fill_q.append(lambda s=s: proj_v(s))
                  nkb = 4 * qt + 4
                  nsteps = 2 * nkb
                  step = 0
                  for pair in range(2):
                      qs, ks = pair, 2 + pair
                      # Interleaved accumulation GROUPS in one PSUM bank are
                      # broken on HW: start=True zeroes the WHOLE bank (HW
                      # verified). Exploit that: the first PV write of each
                      # h-bank (kb=0, qb=0) runs with start=True to zero the
                      # bank, everything else accumulates with start=False.
                      attps = [psum.tile([128, 4, DK + 1], F32, tag="att",
                                         bufs=2, name=f"attps{h}_{qt}_{pair}_{_it}")
                               for h in range(2)]

                      def pv(kb):
                          for qb in range(max(kb - 4 * qt, 0), 4):
                              gq = 4 * qt + qb
                              for h in range(2):
                                  nc.tensor.matmul(
                                      attps[h][:, qb, :],
                                      pab_ring[kb % 4][:, h, qb * 128:(qb + 1) * 128],
                                      vt[:, kb, 2 * pair + h, :],
                                      start=(kb == 0 and qb == 0),
                                      stop=(kb == gq),
                                      skip_group_check=True)

                      att_sb = apool.tile([128, 4, 128], BF16, tag="attsb")
                      rsum = wpool.tile([128, 4, 2], F32, tag="rsum")
                      last = qt == SB - 1 and pair == 1

                      def normalize(qb):
                          # DVE reciprocal of the col-64 sums + DVE broadcast
                          # multiply (gpsimd cannot read PSUM). Per-q-block
                          # only for the very last pair (to pipeline the
                          # tail); batched per-pair otherwise to keep the DVE
                          # instruction count down. Transposes go to the FRONT
                          # of the queue: they are small, release the shared
                          # "sc" PSUM ring fast, and feed the output
                          # projection.
                          if not last:
                              if qb < 3:
                                  return
                              qsl3, nq = slice(0, 4), 4
                          else:
                              qsl3, nq = slice(qb, qb + 1), 1
                          for h in range(2):
                              nc.vector.reciprocal(rsum[:, qsl3, h:h + 1],
                                                   attps[h][:, qsl3, DK:DK + 1])
                              nc.vector.tensor_tensor(
                                  att_sb[:, qsl3, h * 64:(h + 1) * 64],
                                  attps[h][:, qsl3, 0:DK],
                                  rsum[:, qsl3, h:h + 1].to_broadcast(
                                      [128, nq, DK]), MULT)
                          if last:
                              # last qt: output projection chases each q-block
                              for u in reversed(oproj_units(4 * qt + qb,
                                                            split_copy=True)):
                                  fill_q.appendleft(u)
                              fill_q.appendleft(
                                  transpose_unit(4 * qt + qb, pair, att_sb, qb))
                          else:
                              for b in range(3, -1, -1):
                                  fill_q.appendleft(
                                      transpose_unit(4 * qt + b, pair, att_sb, b))

                      pab_ring = {}
                      for kb in range(nkb):
                          lam = max(kb - 4 * qt, 0) * 128
                          qsl = slice(qt * 512 + lam, (qt + 1) * 512)
                          ksl = slice(kb * 128, (kb + 1) * 128)
                          ss = psum.tile([128, 2, 512], F32, tag="sc2", bufs=2)
                          nc.tensor.matmul(ss[:, 0, lam:512], qkT[0:64, ks, ksl],
                                           qkT[0:64, qs, qsl], start=True, stop=True)
                          nc.tensor.matmul(ss[:, 1, lam:512], qkT[64:128, ks, ksl],
                                           qkT[64:128, qs, qsl], start=True, stop=True)
                          pab = ppool.tile([128, 2, 512], BF16, tag="probs")
                          pab_ring[kb % 4] = pab
                          nc.scalar.activation(pab[:, :, lam:512], ss[:, :, lam:512], AF.Exp, scale=0.125)
                          if kb >= 4 * qt:  # diagonal block: causal tri mask
                              dsl = slice(lam, lam + 128)
                              nc.gpsimd.tensor_tensor(
                                  pab[:, :, dsl], pab[:, :, dsl],
                                  tri_sb[:, None, :].to_broadcast([128, 2, 128]), MULT)
                          # drain queued PE work evenly across this qt's steps;
                          # late (output-projection) work backfills in the
                          # ACT-deficit qts
                          n = -(-len(fill_q) // (nsteps - step)) if fill_q else 0
                          if kb >= 4 * qt:
                              n = max(n, 2)  # diag steps absorb more filler
                          if qt >= SB - 2:
                              n = max(n, 1)
                          pump(n, late_ok=(qt >= SB - 2))
                          step += 1
                          # software pipeline: PV for the previous kb runs
                          # after this kb's scores are already in flight
                          if kb > 0:
                              pv(kb - 1)
                              if kb - 1 >= 4 * qt:  # that region just stopped
                                  normalize(kb - 1 - 4 * qt)
                      pv(nkb - 1)
                      normalize(3)
                  if qt < SB - 1:
                      for qb in range(4 * qt, 4 * qt + 4):
                          late_q.extend(oproj_units(qb))
              while late_q:
                  late_q.popleft()()
              while fill_q:
                  fill_q.popleft()()

    nc.compile()
    return nc


def _host_tables(token_positions):
    pos = np.asarray(token_positions, dtype=np.float32)  # [S]
    half = DK // 2
    freq = THETA ** (-np.arange(0, DK, 2, dtype=np.float32) / DK)  # [32]
    # per-partition tables on [dk(128 = 2 heads of 64), s]
    f64 = np.repeat(freq, 2)          # [64] freq per feature index
    ang64 = pos[None, :] * f64[:, None]  # [64, S]
    cos64 = np.cos(ang64)
    sin64 = np.sin(ang64)
    sign = np.where(np.arange(DK) % 2 == 0, 1.0, -1.0).astype(np.float32)  # +s even, -s odd
    sins64 = sin64 * sign[:, None]
    from ml_dtypes import bfloat16 as bf16
    cosf = np.concatenate([cos64, cos64], axis=0).astype(bf16)   # [128, S]
    sins = np.concatenate([sins64, sins64], axis=0).astype(bf16)  # [128, S]
    return cosf, sins


def kernel(x, Wq, Wk, Wv, Wo, token_positions):
    from ml_dtypes import bfloat16 as bf16
    x = np.asarray(x, dtype=np.float32)
    Wq = np.asarray(Wq, dtype=np.float32)
    Wk = np.asarray(Wk, dtype=np.float32)
    Wv = np.asarray(Wv, dtype=np.float32)
    Wo = np.asarray(Wo, dtype=np.float32)

    if "nc" not in _CACHED:
        _CACHED["nc"] = _build_nc(iters=int(os.environ.get("BENCH_ITERS", "1")))
    nc = _CACHED["nc"]

    cosf, sins = _host_tables(token_positions)
    tri = np.triu(np.ones((128, 128), dtype=bf16))  # tri[k, j] = 1 if j >= k
    ident = np.eye(128, dtype=bf16)
    onesc = np.ones((128, 1), dtype=np.float32)

    xT = [np.ascontiguousarray(x[b].T).astype(bf16) for b in range(B)]  # [D, S]
    in_maps = []
    for c in range(NCORES):
        b, g = c // GROUPS, c % GROUPS
        R = slice(g * GF, (g + 1) * GF)
        wqkT = np.ascontiguousarray(
            np.concatenate([Wq[R].T, Wk[R].T], axis=1)).astype(bf16)  # [D, 512]
        wvT = np.ascontiguousarray(Wv[R].T).astype(bf16)              # [D, 256]
        woT = np.ascontiguousarray(Wo[:, R].T).astype(bf16)           # [256, D]
        in_maps.append({
            "xT": xT[b], "wqkT": wqkT, "wvT": wvT, "woT": woT,
            "cosf": cosf, "sins": sins, "tri": tri, "ident": ident, "onesc": onesc,
            "cachebust": np.zeros((int(os.environ.get("BENCH_ITERS", "1")), KVER), dtype=np.float32),
        })

    try:
        res = run_bass_kernel_spmd(nc, in_maps, core_ids=list(range(NCORES)))
    except Exception:
        # transient NRT_EXEC_UNIT_UNRECOVERABLE flakes recover on retry
        import time as _time
        _time.sleep(2.0)
        res = run_bass_kernel_spmd(nc, in_maps, core_ids=list(range(NCORES)))
    _CACHED["last_results"] = res
    outs = [np.asarray(r["out"], dtype=np.float32) for r in res.results]  # [S, D]
    full = np.empty((B, S, D), dtype=np.float32)
    for b in range(B):
        full[b] = sum(outs[b * GROUPS + g] for g in range(GROUPS))
    return full



# revision 90
# speedup vs baseline: 1.3338x; 1.0027x over previous
"""Causal MHA with RoPE on 8 TRN2 NeuronCores.

Sharding: data-parallel over batch (2) x tensor-parallel over heads (4 groups
of 4 heads) = 8 cores. Core c handles batch c//4, head group c%4.
Each core computes its 4 heads' attention and a partial output projection
(Wo sharded row-wise); host sums the 4 partials per batch.

Per-core device algorithm (all matmuls in float32r = TF32, fp32 accumulate):
  - QK^T projection: qkT[dk, s] = (Wqk rows).T-contracted with xT (host-transposed x)
  - RoPE applied on [dk(partition), s] layout via cos/sin tables and a
    stream_shuffle partition pair-swap
  - scores^T[k, q] = K^T.T-free @ Q^T per head (K=64 contraction, two heads
    packed in row groups 0-1 / 2-3 of the PE array)
  - probsT = exp(scores/8) straight from PSUM (no max subtraction; scores are
    N(0,1)-scaled so exp never overflows), causal tri-mask on diagonal tiles
  - attnT_unnorm[dk, q] (+ row of sums via a ones column in [V|1]) = [V|1].T @ probsT
  - softmax normalization: sums row -> K=1 broadcast matmul -> reciprocal ->
    one elementwise multiply
  - partial out = attnT.T-contracted with WoT chunks, accumulated over the
    2 head pairs, DMA'd to DRAM
"""
import sys
import os

for _p in ("/opt/trn_rl_repo", "/root/.axon_site/_ro/trn_rl_repo"):
    if os.path.isdir(_p) and _p not in sys.path:
        sys.path.insert(0, _p)

import numpy as np

import concourse.mybir as mybir
import concourse.tile as tile
from concourse import bacc
from concourse.bass_utils import run_bass_kernel_spmd

F32 = mybir.dt.float32
F32R = mybir.dt.float32r
BF16 = mybir.dt.bfloat16
AF = mybir.ActivationFunctionType
MULT = mybir.AluOpType.mult
ADD = mybir.AluOpType.add
DIV = mybir.AluOpType.divide

B, S, D = 2, 2048, 1024
H, DK = 16, 64
THETA = 10000.0
NCORES = 8
GROUPS = 4          # head groups (tensor parallel)
GH = H // GROUPS    # heads per group = 4
GF = GH * DK        # features per group = 256
SWAP_MASK = [i ^ 1 for i in range(32)]
KVER = 16  # bump on any kernel change: busts the HLO-shape-keyed NEFF cache

_CACHED = {}


def _build_nc(iters=1):
    _iters = iters
    nc = bacc.Bacc("TRN2", target_bir_lowering=False, debug=False, num_devices=NCORES)
    xT = nc.dram_tensor("xT", [D, S], BF16, kind="ExternalInput").ap()
    wqkT = nc.dram_tensor("wqkT", [D, 2 * GF], BF16, kind="ExternalInput").ap()
    wvT = nc.dram_tensor("wvT", [D, GF], BF16, kind="ExternalInput").ap()
    woT = nc.dram_tensor("woT", [GF, D], BF16, kind="ExternalInput").ap()
    cosf = nc.dram_tensor("cosf", [128, S], BF16, kind="ExternalInput").ap()
    sins = nc.dram_tensor("sins", [128, S], BF16, kind="ExternalInput").ap()
    tri = nc.dram_tensor("tri", [128, 128], BF16, kind="ExternalInput").ap()
    ident = nc.dram_tensor("ident", [128, 128], BF16, kind="ExternalInput").ap()
    onesc = nc.dram_tensor("onesc", [128, 1], F32R, kind="ExternalInput").ap()
    # unused input whose shape encodes the kernel version: the neuron compile
    # cache keys on HLO structure only, so two kernels with identical I/O
    # shapes would otherwise collide.
    nc.dram_tensor("cachebust", [iters, KVER], F32, kind="ExternalInput")
    out = nc.dram_tensor("out", [S, D], BF16, kind="ExternalOutput").ap()

    SB = S // 512  # 4 q-tiles of 512
    KB = S // 128  # 16 k-blocks of 128

    with tile.TileContext(nc) as tc:
        with tc.tile_pool(name="const", bufs=1) as cpool, \
             tc.tile_pool(name="big", bufs=1) as bpool, \
             tc.tile_pool(name="work", bufs=2) as wpool, \
             tc.tile_pool(name="asb", bufs=3) as apool, \
             tc.tile_pool(name="probs", bufs=4) as ppool, \
             tc.tile_pool(name="obuf", bufs=3) as opool, \
             tc.tile_pool(name="psum", bufs=1, space="PSUM") as psum:

            # ---- loads, ordered by first use on the single HWDGE queue ----
            wqk_sb = cpool.tile([128, 8, 2 * GF], BF16, tag="wqk")
            wv_sb = cpool.tile([128, 8, GF], BF16, tag="wv")
            wo_sb = cpool.tile([128, 2, D], BF16, tag="wo")
            cos_sb = cpool.tile([128, S], BF16, tag="cos")
            sin_sb = cpool.tile([128, S], BF16, tag="sin")
            tri_sb = cpool.tile([128, 128], BF16, tag="tri")
            ident_sb = cpool.tile([128, 128], BF16, tag="ident")
            onesc_sb = cpool.tile([128, 1], F32R, tag="onesc")
            xt_all = cpool.tile([128, 8, S], BF16, tag="xt")

            def xt_load(tsl, nway=2, eng=None):
                # x feature-chunks dc packed on partitions, batched DMAs
                w = 8 // nway
                for i in range(nway):
                    (eng or nc.sync).dma_start(
                        xt_all[:, i * w:(i + 1) * w, tsl],
                        xT[i * w * 128:(i + 1) * w * 128, tsl].rearrange(
                            "(dc p) s -> p dc s", p=128))

            def wqk_load(c):
                nc.sync.dma_start(
                    wqk_sb[:, :, c * 128:(c + 1) * 128],
                    wqkT[:, c * 128:(c + 1) * 128].rearrange(
                        "(dc p) n -> p dc n", p=128))

            # first tile's inputs go down two DMA queues in parallel:
            # weights on the SP queue, x chunks on the DVE queue
            for halfc in range(2):
                nc.sync.dma_start(
                    wqk_sb[:, halfc * 4:(halfc + 1) * 4, 0:128],
                    wqkT[halfc * 512:(halfc + 1) * 512, 0:128].rearrange(
                        "(dc p) n -> p dc n", p=128))
            xt_load(slice(0, 512), nway=4)
            nc.sync.dma_start(wv_sb[:], wvT.rearrange("(dc p) n -> p dc n", p=128))
            wqk_load(2)
            nc.sync.dma_start(onesc_sb[:], onesc)
            nc.sync.dma_start(cos_sb[:, 0:1024], cosf[:, 0:1024])
            nc.sync.dma_start(sin_sb[:, 0:1024], sins[:, 0:1024])
            wqk_load(1)
            wqk_load(3)
            nc.sync.dma_start(tri_sb[:], tri)
            xt_load(slice(512, 1024))
            nc.sync.dma_start(ident_sb[:], ident)
            nc.sync.dma_start(cos_sb[:, 1024:S], cosf[:, 1024:S])
            nc.sync.dma_start(sin_sb[:, 1024:S], sins[:, 1024:S])
            nc.sync.dma_start(wo_sb[:], woT.rearrange("(fc p) n -> p fc n", p=128))
            xt_load(slice(1024, S))

            warm = cpool.tile([1, 1], F32, tag="warm")
            nc.scalar.activation(warm[:], onesc_sb[0:1, 0:1], AF.Exp, scale=1.0)

            # ---- kernel body ----
            # Projections (phase 1) for x-tile t+1 are interleaved INTO the
            # attention kb-loop over qt=t: attention is ACT(exp)-bound, so the
            # PE fills its idle slots with the next tile's QK/V projections.
            for _it in range(iters):
              qkT = bpool.tile([128, 4, S], BF16, tag="qkT", name=f"qkT{_it}")
              vt = bpool.tile([128, KB, GH, DK + 1], BF16, tag="vt", name=f"vt{_it}")
              nc.vector.tensor_copy(
                  vt[:, :, :, DK:DK + 1],
                  onesc_sb[:, None, None, :].to_broadcast([128, KB, GH, 1]))

              def proj_qk_half(t, c, half, ps):
                  for dc in range(4 * half, 4 * half + 4):
                      nc.tensor.matmul(
                          ps[:], wqk_sb[:, dc, c * 128:(c + 1) * 128],
                          xt_all[:, dc, t * 512:(t + 1) * 512],
                          start=(dc == 0), stop=(dc == 7))

              def proj_qk(t, c, fast=False, half=None):
                  # QK projection chunk: 128 features (head pair c of Q|K),
                  # 512 seq positions, full D contraction; then RoPE.
                  ps = psum.tile([128, 512], F32, tag="sc", bufs=2)
                  proj_qk_half(t, c, 0, ps)
                  if half is not None:
                      # second half (+ RoPE) deferred as the next filler unit
                      half.appendleft(lambda: proj_qk_rope(t, c, ps, fast))
                      return
                  proj_qk_rope(t, c, ps, fast)

              def proj_qk_rope(t, c, ps, fast=False):
                  proj_qk_half(t, c, 1, ps)
                  tsl = slice(t * 512, (t + 1) * 512)
                  # rope: qkT = ps*cos + swap(ps*sins), all on DVE (Pool is
                  # reserved for the latency-critical causal masks)
                  if fast:
                      # prologue chunks: pre-round ps to bf16 on the (idle)
                      # ACT engine so the DVE multiplies run in 2x mode
                      psb = wpool.tile([128, 512], BF16, tag="psb")
                      nc.scalar.copy(psb[:], ps[:])
                      src = psb
                  else:
                      src = ps
                  tmp = wpool.tile([128, 512], BF16, tag="ropetmp")
                  nc.vector.tensor_tensor(tmp[:], src[:], sin_sb[:, tsl], MULT)
                  tmp2 = wpool.tile([128, 512], BF16, tag="ropetmp2")
                  nc.vector.stream_shuffle(tmp2[:], tmp[:], SWAP_MASK)
                  nc.vector.tensor_tensor(qkT[:, c, tsl], src[:], cos_sb[:, tsl], MULT)
                  nc.vector.tensor_tensor(qkT[:, c, tsl], qkT[:, c, tsl], tmp2[:], ADD)

              def proj_v(sb_i, on_act=False):
                  psv = psum.tile([128, GF], F32, tag="sc", bufs=2)
                  for dc in range(8):
                      nc.tensor.matmul(
                          psv[:], xt_all[:, dc, sb_i * 128:(sb_i + 1) * 128],
                          wv_sb[:, dc, :], start=(dc == 0), stop=(dc == 7))
                  if on_act:
                      nc.scalar.copy(vt[:, sb_i, :, 0:DK],
                                     psv[:].rearrange("p (h d) -> p h d", h=GH))
                  else:
                      nc.vector.tensor_copy(
                          vt[:, sb_i, :, 0:DK],
                          psv[:].rearrange("p (h d) -> p h d", h=GH))

              # Deferred-PE-work queue: projection chunks for tile t+1,
              # transposes of the previous pair, and the previous qt's output
              # projection all get pumped into the attention kb-loop so the
              # (in-order) PE never sits behind a dependency-stalled
              # instruction for long.
              from collections import deque
              fill_q = deque()   # prompt PE work (projections, transposes)
              late_q = deque()   # output projections, deferred to late qts
                                 # where attention has an ACT-vs-PE deficit

              def pump(n=1, late_ok=False):
                  for _ in range(n):
                      if fill_q:
                          fill_q.popleft()()
                      elif late_ok and late_q:
                          late_q.popleft()()

              def transpose_unit(gq, pair, att_sb, qb):
                  def run():
                      tps = psum.tile([128, 128], BF16, tag="sc", bufs=2,
                                      name=f"tps{gq}_{pair}_{_it}")
                      nc.tensor.transpose(tps[:], att_sb[:, qb, :], ident_sb[:])
                      nc.vector.tensor_copy(
                          attnT[pair][:, gq * 128:(gq + 1) * 128], tps[:])
                  return run

              def oproj_units(qb, split_copy=False):
                  # output projection for one 128-q-block, as two filler units
                  # (one per 512-wide n-half; the second issues the DMA)
                  st = {}

                  def run_nh(nh):
                      qsl = slice(qb * 128, (qb + 1) * 128)
                      if nh == 0:
                          st["osb"] = opool.tile([128, D], BF16, tag="osb",
                                                 name=f"osb{qb}_{_it}")
                      osb = st["osb"]
                      nsl = slice(nh * 512, (nh + 1) * 512)
                      pso = psum.tile([128, 512], F32, tag="sc", bufs=2)
                      nc.tensor.matmul(pso[:], attnT[0][:, qsl],
                                       wo_sb[:, 0, nsl], start=True, stop=False)
                      nc.tensor.matmul(pso[:], attnT[1][:, qsl],
                                       wo_sb[:, 1, nsl], start=False, stop=True)
                      if split_copy and nh == 1:
                          nc.scalar.copy(osb[:, nsl], pso[:])
                      else:
                          nc.vector.tensor_copy(osb[:, nsl], pso[:])
                      if split_copy:
                          # last qt: one DMA per n-half so the final transfer
                          # is half-size
                          nc.sync.dma_start(out[qsl, nsl], osb[:, nsl])
                      elif nh == 1:
                          nc.sync.dma_start(out[qsl, :], osb[:])

                  return [lambda: run_nh(0), lambda: run_nh(1)]

              # ---- attention (PV-flipped), deferred work interleaved ----
              # PV: attn[q, dk] = probsT.T @ [V|1] per 128-q-block: N=65 moving
              # rows instead of N=512, fully using the 128-wide K (k-positions)
              # and M (q) dims of the PE array. Softmax sums land in column 64
              # as per-partition scalars -> normalization via Pool broadcast
              # multiply, then a PE transpose restores [f, q] layout for the
              # output projection.
              attnT = [bpool.tile([128, S], BF16, tag=f"attnT{p}",
                                  name=f"attnT{p}_{_it}") for p in range(2)]

              # tile t=0: pair-0's needs (Q01, K01, V) up front; Q23/K23 queued.
              # fast=True / on_act=True shift prologue elementwise work onto
              # the idle ACT engine to shorten the first-attention latency.
              proj_qk(0, 0, fast=True)
              proj_qk(0, 2, fast=True)
              for s in range(4):
                  proj_v(s, on_act=True)
              fill_q.append(lambda: proj_qk(0, 1, fast=True))
              fill_q.append(lambda: proj_qk(0, 3, fast=True))

              state = {"att_next": None, "seq": 0}
              for qt in range(SB):
                  if qt + 1 < SB:
                      t = qt + 1
                      for c in range(4):
                          fill_q.append(lambda t=t, c=c: proj_qk(t, c, fast=True, half=fill_q))
                      for s in range(4 * t, 4 * t + 4):
                          fill_q.append(lambda s=s: proj_v(s))
                  nkb = 4 * qt + 4
                  nsteps = 2 * nkb
                  step = 0
                  for pair in range(2):
                      qs, ks = pair, 2 + pair
                      # Interleaved accumulation GROUPS in one PSUM bank are
                      # broken on HW: start=True zeroes the WHOLE bank (HW
                      # verified). Exploit that: the first PV write of each
                      # h-bank (kb=0, qb=0) runs with start=True to zero the
                      # bank, everything else accumulates with start=False.
                      attps = [psum.tile([128, 4, DK + 1], F32, tag="att",
                                         bufs=2, name=f"attps{h}_{qt}_{pair}_{_it}")
                               for h in range(2)]

                      def pv(kb):
                          for qb in range(max(kb - 4 * qt, 0), 4):
                              gq = 4 * qt + qb
                              for h in range(2):
                                  nc.tensor.matmul(
                                      attps[h][:, qb, :],
                                      pab_ring[kb % 4][:, h, qb * 128:(qb + 1) * 128],
                                      vt[:, kb, 2 * pair + h, :],
                                      start=(kb == 0 and qb == 0),
                                      stop=(kb == gq),
                                      skip_group_check=True)

                      att_sb = apool.tile([128, 4, 128], BF16, tag="attsb")
                      rsum = wpool.tile([128, 4, 2], F32, tag="rsum")
                      last = qt == SB - 1 and pair == 1

                      def normalize(qb):
                          # DVE reciprocal of the col-64 sums + DVE broadcast
                          # multiply (gpsimd cannot read PSUM). Per-q-block
                          # only for the very last pair (to pipeline the
                          # tail); batched per-pair otherwise to keep the DVE
                          # instruction count down. Transposes go to the FRONT
                          # of the queue: they are small, release the shared
                          # "sc" PSUM ring fast, and feed the output
                          # projection.
                          if not last:
                              if qb < 3:
                                  return
                              qsl3, nq = slice(0, 4), 4
                          else:
                              qsl3, nq = slice(qb, qb + 1), 1
                          for h in range(2):
                              nc.vector.reciprocal(rsum[:, qsl3, h:h + 1],
                                                   attps[h][:, qsl3, DK:DK + 1])
                              nc.vector.tensor_tensor(
                                  att_sb[:, qsl3, h * 64:(h + 1) * 64],
                                  attps[h][:, qsl3, 0:DK],
                                  rsum[:, qsl3, h:h + 1].to_broadcast(
                                      [128, nq, DK]), MULT)
                          if last:
                              # last qt: output projection chases each q-block
                              for u in reversed(oproj_units(4 * qt + qb,
                                                            split_copy=True)):
                                  fill_q.appendleft(u)
                              fill_q.appendleft(
                                  transpose_unit(4 * qt + qb, pair, att_sb, qb))
                          else:
                              for b in range(3, -1, -1):
                                  fill_q.appendleft(
                                      transpose_unit(4 * qt + b, pair, att_sb, b))

                      pab_ring = {}
                      for kb in range(nkb):
                          lam = max(kb - 4 * qt, 0) * 128
                          qsl = slice(qt * 512 + lam, (qt + 1) * 512)
                          ksl = slice(kb * 128, (kb + 1) * 128)
                          ss = psum.tile([128, 2, 512], F32, tag="sc2", bufs=2)
                          nc.tensor.matmul(ss[:, 0, lam:512], qkT[0:64, ks, ksl],
                                           qkT[0:64, qs, qsl], start=True, stop=True)
                          nc.tensor.matmul(ss[:, 1, lam:512], qkT[64:128, ks, ksl],
                                           qkT[64:128, qs, qsl], start=True, stop=True)
                          pab = ppool.tile([128, 2, 512], BF16, tag="probs")
                          pab_ring[kb % 4] = pab
                          nc.scalar.activation(pab[:, :, lam:512], ss[:, :, lam:512], AF.Exp, scale=0.125)
                          if kb >= 4 * qt:  # diagonal block: causal tri mask
                              # on DVE: all-bf16 SBUF operands hit 2x mode,
                              # shortening the exp->mask->PV diagonal chain
                              dsl = slice(lam, lam + 128)
                              nc.vector.tensor_tensor(
                                  pab[:, :, dsl], pab[:, :, dsl],
                                  tri_sb[:, None, :].to_broadcast([128, 2, 128]), MULT)
                          # drain queued PE work evenly across this qt's steps;
                          # late (output-projection) work backfills in the
                          # ACT-deficit qts
                          n = -(-len(fill_q) // (nsteps - step)) if fill_q else 0
                          if kb >= 4 * qt:
                              n = max(n, 2)  # diag steps absorb more filler
                          if qt >= SB - 2:
                              n = max(n, 1)
                          pump(n, late_ok=(qt >= SB - 2))
                          step += 1
                          # software pipeline: PV for the previous kb runs
                          # after this kb's scores are already in flight
                          if kb > 0:
                              pv(kb - 1)
                              if kb - 1 >= 4 * qt:  # that region just stopped
                                  normalize(kb - 1 - 4 * qt)
                      pv(nkb - 1)
                      normalize(3)
                  if qt < SB - 1:
                      for qb in range(4 * qt, 4 * qt + 4):
                          late_q.extend(oproj_units(qb))
              while late_q:
                  late_q.popleft()()
              while fill_q:
                  fill_q.popleft()()

    nc.compile()
    return nc


def _host_tables(token_positions):
    pos = np.asarray(token_positions, dtype=np.float32)  # [S]
    half = DK // 2
    freq = THETA ** (-np.arange(0, DK, 2, dtype=np.float32) / DK)  # [32]
    # per-partition tables on [dk(128 = 2 heads of 64), s]
    f64 = np.repeat(freq, 2)          # [64] freq per feature index
    ang64 = pos[None, :] * f64[:, None]  # [64, S]
    cos64 = np.cos(ang64)
    sin64 = np.sin(ang64)
    sign = np.where(np.arange(DK) % 2 == 0, 1.0, -1.0).astype(np.float32)  # +s even, -s odd
    sins64 = sin64 * sign[:, None]
    from ml_dtypes import bfloat16 as bf16
    cosf = np.concatenate([cos64, cos64], axis=0).astype(bf16)   # [128, S]
    sins = np.concatenate([sins64, sins64], axis=0).astype(bf16)  # [128, S]
    return cosf, sins


def kernel(x, Wq, Wk, Wv, Wo, token_positions):
    from ml_dtypes import bfloat16 as bf16
    x = np.asarray(x, dtype=np.float32)
    Wq = np.asarray(Wq, dtype=np.float32)
    Wk = np.asarray(Wk, dtype=np.float32)
    Wv = np.asarray(Wv, dtype=np.float32)
    Wo = np.asarray(Wo, dtype=np.float32)

    if "nc" not in _CACHED:
        _CACHED["nc"] = _build_nc(iters=int(os.environ.get("BENCH_ITERS", "1")))
    nc = _CACHED["nc"]

    cosf, sins = _host_tables(token_positions)
    tri = np.triu(np.ones((128, 128), dtype=bf16))  # tri[k, j] = 1 if j >= k
    ident = np.eye(128, dtype=bf16)
    onesc = np.ones((128, 1), dtype=np.float32)

    xT = [np.ascontiguousarray(x[b].T).astype(bf16) for b in range(B)]  # [D, S]
    in_maps = []
    for c in range(NCORES):
        b, g = c // GROUPS, c % GROUPS
        R = slice(g * GF, (g + 1) * GF)
        wqkT = np.ascontiguousarray(
            np.concatenate([Wq[R].T, Wk[R].T], axis=1)).astype(bf16)  # [D, 512]
        wvT = np.ascontiguousarray(Wv[R].T).astype(bf16)              # [D, 256]
        woT = np.ascontiguousarray(Wo[:, R].T).astype(bf16)           # [256, D]
        in_maps.append({
            "xT": xT[b], "wqkT": wqkT, "wvT": wvT, "woT": woT,
            "cosf": cosf, "sins": sins, "tri": tri, "ident": ident, "onesc": onesc,
            "cachebust": np.zeros((int(os.environ.get("BENCH_ITERS", "1")), KVER), dtype=np.float32),
        })

    try:
        res = run_bass_kernel_spmd(nc, in_maps, core_ids=list(range(NCORES)))
    except Exception:
        # transient NRT_EXEC_UNIT_UNRECOVERABLE flakes recover on retry
        import time as _time
        _time.sleep(2.0)
        res = run_bass_kernel_spmd(nc, in_maps, core_ids=list(range(NCORES)))
    _CACHED["last_results"] = res
    outs = [np.asarray(r["out"], dtype=np.float32) for r in res.results]  # [S, D]
    full = np.empty((B, S, D), dtype=np.float32)
    for b in range(B):
        full[b] = sum(outs[b * GROUPS + g] for g in range(GROUPS))
    return full

